# revision 1
# baseline (speedup 1.0000x reference)
"""Multi-head causal attention (B=2, S=2048, D=1024, H=16, Dh=64) on 8 TRN2
NeuronCores.

Sharding: core c = 4*b + g handles batch b (2-way data parallel) and head
group g (4-way tensor parallel over the 16 heads: heads 4g..4g+3, i.e. a
256-column slice of W_q/W_k/W_v, and the matching 256-row slice of W_o).
Each core returns a partial output [S, D]; the host sums the 4 partials per
batch and adds b_o (row-parallel out-projection reduce).

On-core layout is "K-major" flash attention: scores are computed transposed
(S^T[k, q] = K q^T) so softmax's sum over k can be folded into the
attn@V matmul by augmenting V's stationary tile with 64 columns of ones
(denominator lands in the other half of the PSUM partition range).
All matmuls run in float32r (full-rate fp32 on the PE array).
"""

import numpy as np
from contextlib import ExitStack

import concourse.bass as bass
import concourse.bacc as bacc
import concourse.tile as tile
import concourse.mybir as mybir
from concourse.bass_utils import run_bass_kernel_spmd

F32 = mybir.dt.float32
F32R = mybir.dt.float32r
AF = mybir.ActivationFunctionType

B = 2
S = 2048
D = 1024
DC = 256  # head dims per core (4 heads x 64)
N_CORES = 8
NT = D // 128  # 8 input-dim tiles
ST = S // 128  # 16 sequence tiles


def _slices512(off, end):
    """Bank-aligned column slices of [off, end): split at 512 boundaries."""
    out = []
    a = off
    while a < end:
        b = min(end, (a // 512 + 1) * 512)
        out.append((a, b))
        a = b
    return out


def _build():
    nc = bacc.Bacc("TRN2", target_bir_lowering=False, debug=False,
                   num_devices=N_CORES)
    xt = nc.dram_tensor("xt", [D, S], F32R, kind="ExternalInput").ap()
    wq = nc.dram_tensor("wq", [D, DC], F32R, kind="ExternalInput").ap()
    wk = nc.dram_tensor("wk", [D, DC], F32R, kind="ExternalInput").ap()
    wv = nc.dram_tensor("wv", [D, DC], F32R, kind="ExternalInput").ap()
    wo = nc.dram_tensor("wo", [DC, D], F32R, kind="ExternalInput").ap()
    mk = nc.dram_tensor("mk", [128, 128], F32, kind="ExternalInput").ap()
    y = nc.dram_tensor("y", [S, D], F32, kind="ExternalOutput").ap()

    with tile.TileContext(nc) as tc, ExitStack() as stk:
        persist = stk.enter_context(tc.tile_pool(name="persist", bufs=1))
        qt_sb = persist.tile([128, 2 * S], F32R)   # Q^T/8: dq-tile j at cols 2048j
        kt_sb = persist.tile([128, 2 * S], F32R)   # K^T
        # V per k-tile block of 512 cols: head h sub-block of 128 cols =
        # [V_h | ones] for even h, [ones | V_h] for odd h.
        v_sb = persist.tile([128, ST * 512], F32R)
        ct_sb = persist.tile([128, 2 * S], F32R)   # normalized ctx^T
        wo_sb = persist.tile([128, 2 * D], F32R)   # W_o slice: dc-tile d at cols 1024d
        mk_sb = persist.tile([128, 128], F32)      # mask[k, q] = (k <= q)

        nc.sync.dma_start(out=mk_sb[:], in_=mk[:, :])
        for d in range(2):
            nc.sync.dma_start(out=wo_sb[:, 1024 * d:1024 * (d + 1)],
                              in_=wo[128 * d:128 * (d + 1), :])

        # ---- projections: Q^T, K^T (dq on partitions) and V ----
        with tc.tile_pool(name="stg1", bufs=1) as stg1, \
             tc.tile_pool(name="ppq", bufs=2, space="PSUM") as ppq, \
             tc.tile_pool(name="ppv", bufs=4, space="PSUM") as ppv:
            xt_sb = stg1.tile([128, NT * S], F32R)
            wq_sb = stg1.tile([128, NT * DC], F32R)
            wk_sb = stg1.tile([128, NT * DC], F32R)
            wv_sb = stg1.tile([128, NT * DC], F32R)
            for i in range(NT):
                nc.sync.dma_start(out=xt_sb[:, S * i:S * (i + 1)],
                                  in_=xt[128 * i:128 * (i + 1), :])
                nc.sync.dma_start(out=wq_sb[:, DC * i:DC * (i + 1)],
                                  in_=wq[128 * i:128 * (i + 1), :])
                nc.sync.dma_start(out=wk_sb[:, DC * i:DC * (i + 1)],
                                  in_=wk[128 * i:128 * (i + 1), :])
                nc.sync.dma_start(out=wv_sb[:, DC * i:DC * (i + 1)],
                                  in_=wv[128 * i:128 * (i + 1), :])

            def qk_half(j, w_sb, dst, half):
                ps = ppq.tile([128, 1024], F32, tag="ppq", name=f"pq{j}{half}")
                for i in range(NT):
                    for a, b in ((0, 512), (512, 1024)):
                        nc.tensor.matmul(
                            ps[:, a:b],
                            lhsT=(w_sb[:, DC * i + 128 * j:
                                       DC * i + 128 * (j + 1)]),
                            rhs=(xt_sb[:, S * i + 1024 * half + a:
                                       S * i + 1024 * half + b]),
                            start=(i == 0), stop=(i == NT - 1))
                nc.scalar.copy(
                    dst[:, 2048 * j + 1024 * half:
                        2048 * j + 1024 * (half + 1)], ps[:, :])

            def v_round(st):
                nc.vector.memset(
                    v_sb[:, 512 * st:512 * (st + 1)].bitcast(F32), 1.0)
                pv = ppv.tile([128, 256], F32, tag="ppv", name=f"pv{st}")
                for i in range(NT):
                    nc.tensor.matmul(
                        pv[:, 0:256],
                        lhsT=(xt_sb[:, S * i + 128 * st:
                                    S * i + 128 * (st + 1)]),
                        rhs=(wv_sb[:, DC * i:DC * (i + 1)]),
                        start=(i == 0), stop=(i == NT - 1))
                base = 512 * st
                blk = v_sb[:, base:base + 512].rearrange(
                    "p (h c) -> p h c", c=256)
                srcv = pv[:, 0:256].rearrange("p (h c) -> p h c", c=128)
                nc.vector.tensor_copy(blk[:, :, 0:64], srcv[:, :, 0:64])
                nc.vector.tensor_copy(blk[:, :, 192:256], srcv[:, :, 64:128])

            for half in range(2):
                qk_half(0, wq_sb, qt_sb, half)
            for half in range(2):
                qk_half(0, wk_sb, kt_sb, half)
            for st in range(8):
                v_round(st)
            for half in range(2):
                qk_half(1, wq_sb, qt_sb, half)
            for half in range(2):
                qk_half(1, wk_sb, kt_sb, half)
            for st in range(8, ST):
                v_round(st)

        # ---- attention per (head, 1024-query-chunk) ----
        with tc.tile_pool(name="sp", bufs=2, space="PSUM") as sp, \
             tc.tile_pool(name="cp", bufs=2, space="PSUM") as cp, \
             tc.tile_pool(name="ep", bufs=10) as ep, \
             tc.tile_pool(name="rp", bufs=6) as rp:
            for h in range(4):
                jh = h // 2
                hb = 64 * (h % 2)   # partition base where ctx lands
                dr = 64 - hb        # partition base where denominator lands
                for qc in range(2):
                    ctx_ps = cp.tile([128, 1024], F32, tag="ctx",
                                     name=f"cx{h}{qc}")
                    kt_max = 8 * qc + 7

                    def ctx_round(kt, e_sb, off):
                        for a, b in _slices512(off, 1024):
                            last_kt = 8 * qc + (3 if b <= 512 else 7)
                            nc.tensor.matmul(
                                ctx_ps[:, a:b],
                                lhsT=(v_sb[:, 512 * kt + 128 * h:
                                           512 * kt + 128 * (h + 1)]),
                                rhs=(e_sb[:, a:b]),
                                start=(kt == 0), stop=(kt == last_kt))

                    for kt in range(kt_max + 1):
                        q_lo = max(1024 * qc, 128 * kt)
                        off = q_lo - 1024 * qc
                        s_ps = sp.tile([128, 1024], F32, tag="s",
                                       name=f"s{h}{qc}{kt}")
                        e_sb = ep.tile([128, 1024], F32R, tag="e",
                                       name=f"e{h}{qc}{kt}")
                        for a, b in _slices512(off, 1024):
                            # fp32r needs a >=256-wide moving operand for
                            # full rate; widen narrow leading slices downward
                            # (extra cols land before `off`, never read)
                            a = min(a, b - 256)
                            nc.tensor.matmul(
                                s_ps[:, a:b],
                                lhsT=(kt_sb[hb:hb + 64,
                                            2048 * jh + 128 * kt:
                                            2048 * jh + 128 * (kt + 1)]),
                                rhs=(qt_sb[hb:hb + 64,
                                           2048 * jh + 1024 * qc + a:
                                           2048 * jh + 1024 * qc + b]),
                                start=True, stop=True)
                        nc.scalar.activation(e_sb[:, off:1024],
                                             s_ps[:, off:1024], AF.Exp)
                        if 128 * kt >= 1024 * qc:
                            # diagonal block: zero strictly-lower (k > q)
                            nc.gpsimd.tensor_mul(e_sb[:, off:off + 128],
                                                 e_sb[:, off:off + 128],
                                                 mk_sb[:, :])
                        ctx_round(kt, e_sb, off)

                    # normalize: ctx rows are [hb, hb+64), denominator rows
                    # (sum of exp) are [dr, dr+64), replicated columns.
                    rcp = rp.tile([128, 1024], F32, tag="rcp", name=f"r{h}{qc}")
                    rcb = rp.tile([128, 1024], F32, tag="rcb", name=f"rb{h}{qc}")
                    # NB: reciprocal_approx_* miscompute at partition base != 0
                    nc.vector.reciprocal(rcp[dr:dr + 1, :], ctx_ps[dr:dr + 1, :])
                    if dr == 0:
                        # gpsimd broadcast (reads true partition 0 only)
                        nc.gpsimd.partition_broadcast(rcb[:, :], rcp[0:1, :])
                    else:
                        nc.sync.dma_start(
                            out=rcb[hb:hb + 64, :],
                            in_=rcp[dr:dr + 1, :].unsqueeze(1)
                            .to_broadcast((1, 64, 1024)))
                    nc.vector.tensor_mul(
                        ct_sb[hb:hb + 64,
                              2048 * jh + 1024 * qc:
                              2048 * jh + 1024 * (qc + 1)],
                        ctx_ps[hb:hb + 64, :], rcb[hb:hb + 64, :])

        # ---- partial out-projection y = ctx @ W_o[slice] ----
        with tc.tile_pool(name="op", bufs=2, space="PSUM") as op, \
             tc.tile_pool(name="ob", bufs=6) as ob:
            for st in range(ST):
                o_ps = op.tile([128, 1024], F32, tag="o", name=f"op{st}")
                for d in range(2):
                    for a, b in ((0, 512), (512, 1024)):
                        nc.tensor.matmul(
                            o_ps[:, a:b],
                            lhsT=(ct_sb[:, 2048 * d + 128 * st:
                                        2048 * d + 128 * (st + 1)]),
                            rhs=(wo_sb[:, 1024 * d + a:1024 * d + b]),
                            start=(d == 0), stop=(d == 1))
                o_sb = ob.tile([128, 1024], F32, tag="osb", name=f"ob{st}")
                nc.vector.tensor_copy(o_sb[:, :], o_ps[:, :])
                nc.sync.dma_start(out=y[128 * st:128 * (st + 1), :],
                                  in_=o_sb[:, :])

    nc.compile()
    return nc


_nc = None


def kernel(x, W_q, W_k, W_v, W_o, b_o):
    global _nc
    x = np.ascontiguousarray(np.asarray(x, dtype=np.float32))
    W_q = np.asarray(W_q, dtype=np.float32)
    W_k = np.asarray(W_k, dtype=np.float32)
    W_v = np.asarray(W_v, dtype=np.float32)
    W_o = np.asarray(W_o, dtype=np.float32)
    b_o = np.asarray(b_o, dtype=np.float32)

    if _nc is None:
        _nc = _build()

    mask = np.triu(np.ones((128, 128), dtype=np.float32))  # 1 where k <= q
    in_maps = []
    for c in range(N_CORES):
        b = c // 4
        g = c % 4
        sl = slice(DC * g, DC * (g + 1))
        in_maps.append({
            "xt": np.ascontiguousarray(x[b].T),
            "wq": np.ascontiguousarray(W_q[:, sl]) * 0.125,  # fold 1/sqrt(Dh)
            "wk": np.ascontiguousarray(W_k[:, sl]),
            "wv": np.ascontiguousarray(W_v[:, sl]),
            "wo": np.ascontiguousarray(W_o[sl, :]),
            "mk": mask,
        })

    res = run_bass_kernel_spmd(_nc, in_maps, list(range(N_CORES)))
    parts = [res.results[c]["y"] for c in range(N_CORES)]
    out = np.empty((B, S, D), dtype=np.float32)
    for b in range(B):
        acc = np.zeros((S, D), dtype=np.float64)
        for g in range(4):
            acc += parts[4 * b + g]
        acc += b_o
        out[b] = acc.astype(np.float32)
    return out



# revision 5
# speedup vs baseline: 1.2195x; 1.2195x over previous
"""Multi-head causal attention (B=2, S=2048, D=1024, H=16, Dh=64) on 8 TRN2
NeuronCores.

Sharding: core c = 4*b + g handles batch b (2-way data parallel) and head
group g (4-way tensor parallel over the 16 heads: heads 4g..4g+3, i.e. a
256-column slice of W_q/W_k/W_v, and the matching 256-row slice of W_o).
Each core returns a partial output [S, D]; the host sums the 4 partials per
batch and adds b_o (row-parallel out-projection reduce).

On-core dataflow (all matmul operands bf16, accumulation fp32 in PSUM):
scores are computed transposed (S^T[k, q] = K q^T) so softmax's sum over k
is folded into the attn@V matmul by augmenting V's stationary tile with 64
columns of ones (denominator lands in the other half of the PSUM partition
range). Attention processes head PAIRS with 512-wide q-chunks: one score
PSUM tile holds both heads' scores for a k-tile so a single (strided)
activation computes exp for the pair.  The instruction stream is software-
pipelined: the exp-bound attention windows are back-filled with projection
and out-projection matmuls drained from a filler queue, so the PE stays
busy end to end.
"""

import numpy as np
from collections import deque
from contextlib import ExitStack

import ml_dtypes
import concourse.bass as bass
import concourse.bacc as bacc
import concourse.tile as tile
import concourse.mybir as mybir
from concourse.bass_utils import run_bass_kernel_spmd

F32 = mybir.dt.float32
BF16 = mybir.dt.bfloat16
AF = mybir.ActivationFunctionType

B = 2
S = 2048
D = 1024
DC = 256  # head dims per core (4 heads x 64)
N_CORES = 8
NT = D // 128  # 8 input-dim tiles
ST = S // 128  # 16 sequence tiles
QC = 512  # attention q-chunk
NQC = S // QC  # 4 q-chunks


def _build():
    nc = bacc.Bacc("TRN2", target_bir_lowering=False, debug=False,
                   num_devices=N_CORES)
    xt = nc.dram_tensor("xt", [D, S], BF16, kind="ExternalInput").ap()
    wq = nc.dram_tensor("wq", [D, DC], BF16, kind="ExternalInput").ap()
    wk = nc.dram_tensor("wk", [D, DC], BF16, kind="ExternalInput").ap()
    wv = nc.dram_tensor("wv", [D, DC], BF16, kind="ExternalInput").ap()
    wo = nc.dram_tensor("wo", [DC, D], BF16, kind="ExternalInput").ap()
    mk = nc.dram_tensor("mk", [128, 128], BF16, kind="ExternalInput").ap()
    y = nc.dram_tensor("y", [S, D], F32, kind="ExternalOutput").ap()

    with tile.TileContext(nc) as tc, ExitStack() as stk:
        persist = stk.enter_context(tc.tile_pool(name="persist", bufs=1))
        # Q^T / K^T: j-block (heads 2j, 2j+1) at cols 2048j; head 2j on
        # partitions 0:64, head 2j+1 on 64:128.
        qt_sb = persist.tile([128, 2 * S], BF16)
        kt_sb = persist.tile([128, 2 * S], BF16)
        # V per k-tile block of 512 cols: head h sub-block of 128 cols =
        # [V_h | ones] for even h, [ones | V_h] for odd h.
        v_sb = persist.tile([128, ST * 512], BF16)
        ct_sb = persist.tile([128, 2 * S], BF16)   # normalized ctx^T
        wo_sb = persist.tile([128, 2 * D], BF16)   # W_o slice: d-tile at 1024d
        mk_sb = persist.tile([128, 128], BF16)     # mask[k, q] = (k <= q)
        xt_sb = persist.tile([128, NT * S], BF16)  # x^T: d-tile i at cols 2048i
        wq_sb = persist.tile([128, NT * DC], BF16)
        wk_sb = persist.tile([128, NT * DC], BF16)
        wv_sb = persist.tile([128, NT * DC], BF16)

        # ---- input DMAs (arrival order matters) ----
        def w_in(dst, src):
            nc.sync.dma_start(
                out=dst.rearrange("p (i c) -> p i c", c=DC),
                in_=src.rearrange("(i p) c -> p i c", p=128))

        xt3 = xt.rearrange("(i p) s -> p i s", p=128)
        xs3 = xt_sb.rearrange("p (i s) -> p i s", s=S)
        w_in(wk_sb, wk)
        w_in(wq_sb, wq)
        nc.sync.dma_start(out=xs3[:, 0:4, 0:1024], in_=xt3[:, 0:4, 0:1024])
        nc.sync.dma_start(out=xs3[:, 4:8, 0:1024], in_=xt3[:, 4:8, 0:1024])
        w_in(wv_sb, wv)
        nc.sync.dma_start(out=mk_sb[:], in_=mk[:, :])
        nc.sync.dma_start(out=xs3[:, :, 1024:2048], in_=xt3[:, :, 1024:2048])
        for d in range(2):
            nc.sync.dma_start(out=wo_sb[:, 1024 * d:1024 * (d + 1)],
                              in_=wo[128 * d:128 * (d + 1), :])

        # ones columns of v_sb (static): cols 64:192 of each 256 sub-block
        v3 = v_sb.rearrange("p (n c) -> p n c", c=256)
        nc.vector.memset(v3[:, :, 64:192], 1.0)

        sp = stk.enter_context(tc.tile_pool(name="sp", bufs=2, space="PSUM"))
        cp = stk.enter_context(tc.tile_pool(name="cp", bufs=2, space="PSUM"))
        op = stk.enter_context(tc.tile_pool(name="op", bufs=2, space="PSUM"))
        ep = stk.enter_context(tc.tile_pool(name="ep", bufs=6))
        rp = stk.enter_context(tc.tile_pool(name="rp", bufs=3))
        cs = stk.enter_context(tc.tile_pool(name="cs", bufs=3))
        ob = stk.enter_context(tc.tile_pool(name="ob", bufs=3))

        nid = [0]

        def tag(p):
            nid[0] += 1
            return f"{p}{nid[0]}"

        # ---- projection group emitters (generators; yield = filler step) --
        def qk_group(w_sb, dst, j, sc, big, scalar_copy):
            """Q^T/K^T out rows = dq (j-block), s-cols [512sc, 512sc+w).
            big: [128,1024] psum on tag 's' (pre-attention, w=1024);
            else [128,512] on tag 'o' (filler, w=512)."""
            if big:
                ps = sp.tile([128, 1024], F32, tag="s", name=tag("pq"))
                cols = ((0, 512), (512, 1024))
            else:
                ps = op.tile([128, 512], F32, tag="o", name=tag("pq"))
                cols = ((0, 512),)
            for i in range(NT):
                for a, b in cols:
                    nc.tensor.matmul(
                        ps[:, a:b],
                        lhsT=w_sb[:, DC * i + 128 * j:DC * i + 128 * (j + 1)],
                        rhs=xt_sb[:, S * i + 512 * sc + a:
                                  S * i + 512 * sc + b],
                        start=(i == 0), stop=(i == NT - 1))
                if i % 4 == 3:
                    yield
            w = 1024 if big else 512
            dcol = 2048 * j + 512 * sc
            if scalar_copy:
                nc.scalar.activation(dst[:, dcol:dcol + w], ps[:, 0:w],
                                     AF.Copy)
            else:
                nc.vector.tensor_copy(dst[:, dcol:dcol + w], ps[:, 0:w])
            yield

        def v_group(st, scalar_copy):
            """V block st: out rows = s (128 of st), cols = 256 head dims."""
            ps = op.tile([128, 512], F32, tag="o", name=tag("pv"))
            for i in range(NT):
                nc.tensor.matmul(
                    ps[:, 0:256],
                    lhsT=xt_sb[:, S * i + 128 * st:S * i + 128 * (st + 1)],
                    rhs=wv_sb[:, DC * i:DC * (i + 1)],
                    start=(i == 0), stop=(i == NT - 1))
                if i % 4 == 3:
                    yield
            blk = v3[:, 2 * st:2 * st + 2, :]
            srcv = ps[:, 0:256].rearrange("p (h c) -> p h c", c=128)
            if scalar_copy:
                nc.scalar.activation(blk[:, :, 0:64], srcv[:, :, 0:64],
                                     AF.Copy)
                nc.scalar.activation(blk[:, :, 192:256], srcv[:, :, 64:128],
                                     AF.Copy)
            else:
                nc.vector.tensor_copy(blk[:, :, 0:64], srcv[:, :, 0:64])
                nc.vector.tensor_copy(blk[:, :, 192:256], srcv[:, :, 64:128])
            yield

        def oproj_group(st, half, scalar_copy, o_sb):
            """Out-projection for s-tile st, y-cols half*512."""
            ps = op.tile([128, 512], F32, tag="o", name=tag("po"))
            for d in range(2):
                nc.tensor.matmul(
                    ps[:, 0:512],
                    lhsT=ct_sb[:, 2048 * d + 128 * st:
                               2048 * d + 128 * (st + 1)],
                    rhs=wo_sb[:, 1024 * d + 512 * half:
                              1024 * d + 512 * (half + 1)],
                    start=(d == 0), stop=(d == 1))
            yield
            if scalar_copy:
                nc.scalar.activation(o_sb[:, 512 * half:512 * (half + 1)],
                                     ps[:, 0:512], AF.Copy)
            else:
                nc.vector.tensor_copy(o_sb[:, 512 * half:512 * (half + 1)],
                                      ps[:, 0:512])
            yield

        def oproj_st(st, scalar_copy):
            o_sb = ob.tile([128, 1024], F32, tag="ob", name=tag("ob"))
            yield from oproj_group(st, 0, scalar_copy, o_sb)
            yield from oproj_group(st, 1, scalar_copy, o_sb)
            nc.sync.dma_start(out=y[128 * st:128 * (st + 1), :], in_=o_sb[:])
            yield

        fillers = deque()

        def drain(n=1):
            for _ in range(n):
                advanced = False
                while fillers and not advanced:
                    try:
                        next(fillers[0])
                        advanced = True
                    except StopIteration:
                        fillers.popleft()
                if not advanced:
                    return

        def run_now(gen):
            for _ in gen:
                pass

        # ---- attention for a head pair on one q-chunk ----
        def attention(pair, qc):
            """pair: 0 -> heads 0,1 (j-block 0); 1 -> heads 2,3 (j-block 1).
            q-chunk = [QC*qc, QC*(qc+1))."""
            jb = 2048 * pair
            qb = jb + QC * qc
            kt_max = 4 * qc + 3
            ctx = [cp.tile([128, 512], F32, tag="c", name=tag(f"cx{h}"))
                   for h in range(2)]
            for kt in range(kt_max + 1):
                off = max(0, 128 * kt - QC * qc)
                s_ps = sp.tile([128, 1024], F32, tag="s", name=tag("s"))
                e_sb = ep.tile([128, 1024], BF16, tag="e", name=tag("e"))
                for h in range(2):
                    hb = 64 * h
                    nc.tensor.matmul(
                        s_ps[:, 512 * h + off:512 * (h + 1)],
                        lhsT=kt_sb[hb:hb + 64,
                                   jb + 128 * kt:jb + 128 * (kt + 1)],
                        rhs=qt_sb[hb:hb + 64, qb + off:qb + QC],
                        start=True, stop=True)
                s3 = s_ps.rearrange("p (h c) -> p h c", c=512)
                e3 = e_sb.rearrange("p (h c) -> p h c", c=512)
                nc.scalar.activation(e3[:, :, off:512], s3[:, :, off:512],
                                     AF.Exp)
                if kt >= 4 * qc:
                    # diagonal block: zero strictly-lower (k > q)
                    for h in range(2):
                        nc.vector.tensor_mul(e3[:, h, off:off + 128],
                                             e3[:, h, off:off + 128],
                                             mk_sb[:, :])
                for h in range(2):
                    nc.tensor.matmul(
                        ctx[h][:, off:512],
                        lhsT=v_sb[:, 512 * kt + 128 * (2 * pair + h):
                                  512 * kt + 128 * (2 * pair + h + 1)],
                        rhs=e3[:, h, off:512],
                        start=(kt == 0), stop=(kt == kt_max))
                drain(1)

            # normalize: copy ctx+den to SBUF (frees PSUM), then
            # ct[hb rows] = ctx * (1/den) with den broadcast to hb rows.
            for h in range(2):
                hd = 2 * pair + h
                hb = 64 * h          # ctx rows for head hd in its psum tile
                dr = 64 - hb         # denominator rows
                cd = cs.tile([128, 512], F32, tag="cd", name=tag(f"cd{h}"))
                nc.vector.tensor_copy(cd[:, :], ctx[h][:, :])
                rcp = rp.tile([128, 512], F32, tag="r", name=tag(f"r{h}"))
                rcb = rp.tile([128, 512], F32, tag="rb", name=tag(f"rb{h}"))
                nc.vector.reciprocal(rcp[dr:dr + 1, :], cd[dr:dr + 1, :])
                if dr == 0:
                    # gpsimd broadcast (reads true partition 0 only)
                    nc.gpsimd.partition_broadcast(rcb[:, :], rcp[0:1, :])
                else:
                    nc.sync.dma_start(
                        out=rcb[hb:hb + 64, :],
                        in_=rcp[dr:dr + 1, :].unsqueeze(1)
                        .to_broadcast((1, 64, 512)))
                nc.vector.tensor_mul(
                    ct_sb[hb:hb + 64, qb:qb + QC],
                    cd[hb:hb + 64, :], rcb[hb:hb + 64, :])
                drain(1)

        # ================= emission schedule =================
        # S1: minimum prefix for (pair 0, qc 0): K/Q j0 s0:1024, V st0..3.
        run_now(qk_group(wk_sb, kt_sb, 0, 0, True, True))   # K j0 s0:1024
        run_now(qk_group(wq_sb, qt_sb, 0, 0, True, True))   # Q j0 s0:1024
        for st in range(4):
            run_now(v_group(st, True))

        # registry of remaining projection groups (also queued as fillers).
        kq_gen = {}
        for j in range(2):
            for sc in range(4):
                if j == 0 and sc < 2:
                    continue
                kq_gen[("k", j, sc)] = qk_group(wk_sb, kt_sb, j, sc,
                                                False, False)
                kq_gen[("q", j, sc)] = qk_group(wq_sb, qt_sb, j, sc,
                                                False, False)
        v_gen = {st: v_group(st, False) for st in range(4, 16)}

        # priority order: j1 s0:1024 (pair-1 qc0/1), V st4..7 (qc1),
        # then s1024:2048, then V st8..15.
        for key in (("k", 1, 0), ("q", 1, 0), ("k", 1, 1), ("q", 1, 1)):
            fillers.append(kq_gen[key])
        for st in range(4, 8):
            fillers.append(v_gen[st])
        for sc in (2, 3):
            for j in range(2):
                fillers.append(kq_gen[("k", j, sc)])
                fillers.append(kq_gen[("q", j, sc)])
        for st in range(8, 16):
            fillers.append(v_gen[st])

        def need(pair, qc):
            """Force-emit everything attention(pair, qc) reads."""
            j = pair
            for sc in range(qc + 1):
                if (("k", j, sc)) in kq_gen:
                    run_now(kq_gen[("k", j, sc)])
            if ("q", j, qc) in kq_gen:
                run_now(kq_gen[("q", j, qc)])
            for st in range(4 * qc + 4):
                if st in v_gen:
                    run_now(v_gen[st])

        for qc in range(NQC):
            for pair in range(2):
                need(pair, qc)
                attention(pair, qc)
            if qc < NQC - 1:
                for st in range(4 * qc, 4 * qc + 4):
                    fillers.append(oproj_st(st, False))

        # drain remaining fillers (queued out-proj), then the tail
        # out-projection for the last q-chunk.
        while fillers:
            drain(1)
        tail = deque(oproj_st(st, (st % 2 == 0)) for st in range(12, 16))
        while tail:
            g = tail.popleft()
            try:
                next(g)
                tail.append(g)
            except StopIteration:
                pass

    nc.compile()
    return nc


_nc = None


def make_in_maps(x, W_q, W_k, W_v, W_o):
    """Per-core input dict construction (shared with test.py)."""
    bf = ml_dtypes.bfloat16
    mask = np.triu(np.ones((128, 128), dtype=np.float32))  # 1 where k <= q
    in_maps = []
    for c in range(N_CORES):
        b = c // 4
        g = c % 4
        sl = slice(DC * g, DC * (g + 1))
        in_maps.append({
            "xt": np.ascontiguousarray(x[b].T).astype(bf),
            "wq": (np.ascontiguousarray(W_q[:, sl]) * 0.125).astype(bf),
            "wk": np.ascontiguousarray(W_k[:, sl]).astype(bf),
            "wv": np.ascontiguousarray(W_v[:, sl]).astype(bf),
            "wo": np.ascontiguousarray(W_o[sl, :]).astype(bf),
            "mk": mask.astype(bf),
        })
    return in_maps


def kernel(x, W_q, W_k, W_v, W_o, b_o):
    global _nc
    x = np.ascontiguousarray(np.asarray(x, dtype=np.float32))
    W_q = np.asarray(W_q, dtype=np.float32)
    W_k = np.asarray(W_k, dtype=np.float32)
    W_v = np.asarray(W_v, dtype=np.float32)
    W_o = np.asarray(W_o, dtype=np.float32)
    b_o = np.asarray(b_o, dtype=np.float32)

    if _nc is None:
        _nc = _build()

    in_maps = make_in_maps(x, W_q, W_k, W_v, W_o)
    res = run_bass_kernel_spmd(_nc, in_maps, list(range(N_CORES)))
    parts = [res.results[c]["y"] for c in range(N_CORES)]
    out = np.empty((B, S, D), dtype=np.float32)
    for b in range(B):
        acc = np.zeros((S, D), dtype=np.float64)
        for g in range(4):
            acc += parts[4 * b + g]
        acc += b_o
        out[b] = acc.astype(np.float32)
    return out


# revision 35
# speedup vs baseline: 1.2831x; 1.0522x over previous
"""Multi-head causal attention (B=2, S=2048, D=1024, H=16, Dh=64) on 8 TRN2
NeuronCores.

Sharding: core c = 4*b + g handles batch b (2-way data parallel) and head
group g (4-way tensor parallel over the 16 heads: heads 4g..4g+3, i.e. a
256-column slice of W_q/W_k/W_v, and the matching 256-row slice of W_o).
Each core returns a partial output [S, D]; the host sums the 4 partials per
batch and adds b_o (row-parallel out-projection reduce).

On-core dataflow (all matmul operands bf16, accumulation fp32 in PSUM):
scores are computed transposed (S^T[k, q] = K q^T) so softmax's sum over k
is folded into the attn@V matmul by augmenting V's stationary tile with 64
columns of ones (denominator lands in the other half of the PSUM partition
range). Attention processes head PAIRS with 512-wide q-chunks: one score
PSUM tile holds both heads' scores for a k-tile so a single (strided)
activation computes exp for the pair.  The instruction stream is software-
pipelined: the exp-bound attention windows are back-filled with projection
and out-projection matmuls drained from a filler queue, so the PE stays
busy end to end.
"""

import numpy as np
from collections import deque
from contextlib import ExitStack

import ml_dtypes
import concourse.bass as bass
import concourse.bacc as bacc
import concourse.tile as tile
import concourse.mybir as mybir
from concourse.bass_utils import run_bass_kernel_spmd

F32 = mybir.dt.float32
BF16 = mybir.dt.bfloat16
AF = mybir.ActivationFunctionType

B = 2
S = 2048
D = 1024
DC = 256  # head dims per core (4 heads x 64)
N_CORES = 8
NT = D // 128  # 8 input-dim tiles
ST = S // 128  # 16 sequence tiles
QC = 512  # attention q-chunk
NQC = S // QC  # 4 q-chunks


def _build():
    nc = bacc.Bacc("TRN2", target_bir_lowering=False, debug=False,
                   num_devices=N_CORES)
    xt = nc.dram_tensor("xt", [D, S], BF16, kind="ExternalInput").ap()
    wq = nc.dram_tensor("wq", [D, DC], BF16, kind="ExternalInput").ap()
    wk = nc.dram_tensor("wk", [D, DC], BF16, kind="ExternalInput").ap()
    wv = nc.dram_tensor("wv", [D, DC], BF16, kind="ExternalInput").ap()
    wo = nc.dram_tensor("wo", [DC, D], BF16, kind="ExternalInput").ap()
    mk = nc.dram_tensor("mk", [128, 128], BF16, kind="ExternalInput").ap()
    y = nc.dram_tensor("y", [S, D], BF16, kind="ExternalOutput").ap()

    with tile.TileContext(nc) as tc, ExitStack() as stk:
        persist = stk.enter_context(tc.tile_pool(name="persist", bufs=1))
        # Q^T / K^T: j-block (heads 2j, 2j+1) at cols 2048j; head 2j on
        # partitions 0:64, head 2j+1 on 64:128.
        qt_sb = persist.tile([128, 2 * S], BF16)
        kt_sb = persist.tile([128, 2 * S], BF16)
        # V per k-tile block of 512 cols: head h sub-block of 128 cols =
        # [V_h | ones] for even h, [ones | V_h] for odd h.
        v_sb = persist.tile([128, ST * 512], BF16)
        ct_sb = persist.tile([128, 2 * S], BF16)   # normalized ctx^T
        wo_sb = persist.tile([128, 2 * D], BF16)   # W_o slice: d-tile at 1024d
        mk_sb = persist.tile([128, 128], BF16)     # mask[k, q] = (k <= q)
        xt_sb = persist.tile([128, NT * S], BF16)  # x^T: d-tile i at cols 2048i
        wq_sb = persist.tile([128, NT * DC], BF16)
        wk_sb = persist.tile([128, NT * DC], BF16)
        wv_sb = persist.tile([128, NT * DC], BF16)

        # ---- input DMAs (arrival order matters) ----
        def w_in(dst, src):
            nc.sync.dma_start(
                out=dst.rearrange("p (i c) -> p i c", c=DC),
                in_=src.rearrange("(i p) c -> p i c", p=128))

        xt3 = xt.rearrange("(i p) s -> p i s", p=128)
        xs3 = xt_sb.rearrange("p (i s) -> p i s", s=S)
        w_in(wk_sb, wk)
        nc.sync.dma_start(out=xs3[:, 0:1, 0:1024], in_=xt3[:, 0:1, 0:1024])
        nc.sync.dma_start(out=xs3[:, 1:2, 0:1024], in_=xt3[:, 1:2, 0:1024])
        nc.sync.dma_start(out=xs3[:, 2:4, 0:1024], in_=xt3[:, 2:4, 0:1024])
        w_in(wq_sb, wq)
        nc.sync.dma_start(out=xs3[:, 4:6, 0:1024], in_=xt3[:, 4:6, 0:1024])
        nc.sync.dma_start(out=xs3[:, 6:8, 0:1024], in_=xt3[:, 6:8, 0:1024])
        w_in(wv_sb, wv)
        nc.sync.dma_start(out=mk_sb[:], in_=mk[:, :])
        nc.sync.dma_start(out=xs3[:, :, 1024:2048], in_=xt3[:, :, 1024:2048])
        for d in range(2):
            nc.sync.dma_start(out=wo_sb[:, 1024 * d:1024 * (d + 1)],
                              in_=wo[128 * d:128 * (d + 1), :])

        # ones columns of v_sb (static): cols 64:192 of each 256 sub-block
        v3 = v_sb.rearrange("p (n c) -> p n c", c=256)
        scr_sb = persist.tile([128, 512], BF16)  # warm-up matmul scratch
        nc.vector.memset(scr_sb[:], 0.5)
        nc.vector.memset(v3[:, :, 64:192], 1.0)

        sp = stk.enter_context(tc.tile_pool(name="sp", bufs=2, space="PSUM"))
        cp = stk.enter_context(tc.tile_pool(name="cp", bufs=2, space="PSUM"))
        op = stk.enter_context(tc.tile_pool(name="op", bufs=2, space="PSUM"))
        ep = stk.enter_context(tc.tile_pool(name="ep", bufs=6))
        rp = stk.enter_context(tc.tile_pool(name="rp", bufs=3))
        cs = stk.enter_context(tc.tile_pool(name="cs", bufs=3))
        ob = stk.enter_context(tc.tile_pool(name="ob", bufs=3))

        nid = [0]

        def tag(p):
            nid[0] += 1
            return f"{p}{nid[0]}"

        def warm_pe(n):
            """n throwaway matmuls on scratch data: keeps the PE busy (and
            its p-state ramp warm) across windows where real matmuls are
            blocked on DMA or on a cross-engine chain."""
            ps = sp.tile([128, 1024], F32, tag="s", name=tag("wm"))
            for _ in range(n):
                nc.tensor.matmul(ps[:, 0:512], lhsT=scr_sb[:, 0:128],
                                 rhs=scr_sb[:, 0:512], start=True, stop=True)

        # ---- projection group emitters (generators; yield = filler step) --
        def qk_group(w_sb, dst, j, sc, big, scalar_copy):
            """Q^T/K^T out rows = dq (j-block), s-cols [512sc, 512sc+w).
            big: [128,1024] psum on tag 's' (pre-attention, w=1024);
            else [128,512] on tag 'o' (filler, w=512)."""
            if big:
                ps = sp.tile([128, 1024], F32, tag="s", name=tag("pq"))
                cols = ((0, 512), (512, 1024))
            else:
                ps = op.tile([128, 512], F32, tag="o", name=tag("pq"))
                cols = ((0, 512),)
            for i in range(NT):
                for a, b in cols:
                    nc.tensor.matmul(
                        ps[:, a:b],
                        lhsT=w_sb[:, DC * i + 128 * j:DC * i + 128 * (j + 1)],
                        rhs=xt_sb[:, S * i + 512 * sc + a:
                                  S * i + 512 * sc + b],
                        start=(i == 0), stop=(i == NT - 1))
                if i % 4 == 3:
                    yield
            w = 1024 if big else 512
            dcol = 2048 * j + 512 * sc
            if scalar_copy:
                nc.scalar.activation(dst[:, dcol:dcol + w], ps[:, 0:w],
                                     AF.Copy)
            else:
                nc.vector.tensor_copy(dst[:, dcol:dcol + w], ps[:, 0:w])
            yield

        def v_group(st, scalar_copy):
            """V block st: out rows = s (128 of st), cols = 256 head dims."""
            ps = op.tile([128, 512], F32, tag="o", name=tag("pv"))
            for i in range(NT):
                nc.tensor.matmul(
                    ps[:, 0:256],
                    lhsT=xt_sb[:, S * i + 128 * st:S * i + 128 * (st + 1)],
                    rhs=wv_sb[:, DC * i:DC * (i + 1)],
                    start=(i == 0), stop=(i == NT - 1))
                if i % 4 == 3:
                    yield
            blk = v3[:, 2 * st:2 * st + 2, :]
            srcv = ps[:, 0:256].rearrange("p (h c) -> p h c", c=128)
            if scalar_copy:
                nc.scalar.activation(blk[:, :, 0:64], srcv[:, :, 0:64],
                                     AF.Copy)
                nc.scalar.activation(blk[:, :, 192:256], srcv[:, :, 64:128],
                                     AF.Copy)
            else:
                nc.vector.tensor_copy(blk[:, :, 0:64], srcv[:, :, 0:64])
                nc.vector.tensor_copy(blk[:, :, 192:256], srcv[:, :, 64:128])
            yield

        def oproj_group(st, half, scalar_copy, o_sb):
            """Out-projection for s-tile st, y-cols half*512."""
            ps = op.tile([128, 512], F32, tag="o", name=tag("po"))
            for d in range(2):
                nc.tensor.matmul(
                    ps[:, 0:512],
                    lhsT=ct_sb[:, 2048 * d + 128 * st:
                               2048 * d + 128 * (st + 1)],
                    rhs=wo_sb[:, 1024 * d + 512 * half:
                              1024 * d + 512 * (half + 1)],
                    start=(d == 0), stop=(d == 1))
            yield
            if scalar_copy:
                nc.scalar.activation(o_sb[:, 512 * half:512 * (half + 1)],
                                     ps[:, 0:512], AF.Copy)
            else:
                nc.vector.tensor_copy(o_sb[:, 512 * half:512 * (half + 1)],
                                      ps[:, 0:512])
            yield

        def oproj_st(st, scalar_copy):
            o_sb = ob.tile([128, 1024], BF16, tag="ob", name=tag("ob"))
            yield from oproj_group(st, 0, scalar_copy, o_sb)
            yield from oproj_group(st, 1, scalar_copy, o_sb)
            nc.sync.dma_start(out=y[128 * st:128 * (st + 1), :], in_=o_sb[:])
            yield

        def oproj_big(st, scalar_copy, split=False):
            """Tail-era out-projection: full s-tile on the (now free) 's'
            psum ring.  split: halve the copy across ACT+DVE and the DMA
            across the SP+Pool queues to shorten the final drain."""
            ps = sp.tile([128, 1024], F32, tag="s", name=tag("pt"))
            o_sb = ob.tile([128, 1024], BF16, tag="ob", name=tag("obt"))
            for d in range(2):
                for a, b in ((0, 512), (512, 1024)):
                    nc.tensor.matmul(
                        ps[:, a:b],
                        lhsT=ct_sb[:, 2048 * d + 128 * st:
                                   2048 * d + 128 * (st + 1)],
                        rhs=wo_sb[:, 1024 * d + a:1024 * d + b],
                        start=(d == 0), stop=(d == 1))
            yield
            row = y[128 * st:128 * (st + 1), :]
            if split:
                nc.scalar.activation(o_sb[:, 0:512], ps[:, 0:512], AF.Copy)
                nc.vector.tensor_copy(o_sb[:, 512:1024], ps[:, 512:1024])
                yield
                nc.sync.dma_start(out=row[:, 0:512], in_=o_sb[:, 0:512])
                nc.gpsimd.dma_start(out=row[:, 512:1024],
                                    in_=o_sb[:, 512:1024])
                yield
                return
            if scalar_copy:
                nc.scalar.activation(o_sb[:], ps[:], AF.Copy)
            else:
                nc.vector.tensor_copy(o_sb[:], ps[:])
            yield
            nc.sync.dma_start(out=row[:], in_=o_sb[:])
            yield

        fillers = deque()
        reserve = [0]  # generators held back for the post-attention window

        def drain(n=1):
            for _ in range(n):
                if len(fillers) <= reserve[0]:
                    return
                advanced = False
                while len(fillers) > reserve[0] and not advanced:
                    try:
                        next(fillers[0])
                        advanced = True
                    except StopIteration:
                        fillers.popleft()
                if not advanced:
                    return

        def run_now(gen):
            for _ in gen:
                pass

        # ---- attention for a head pair on one q-chunk ----
        def attention(pair, qc, ndrain=1, direct_norm=False):
            """pair: 0 -> heads 0,1 (j-block 0); 1 -> heads 2,3 (j-block 1).
            q-chunk = [QC*qc, QC*(qc+1))."""
            jb = 2048 * pair
            qb = jb + QC * qc
            kt_max = 4 * qc + 3
            ctx = [cp.tile([128, 512], F32, tag="c", name=tag(f"cx{h}"))
                   for h in range(2)]
            for kt in range(kt_max + 1):
                off = max(0, 128 * kt - QC * qc)
                s_ps = sp.tile([128, 1024], F32, tag="s", name=tag("s"))
                e_sb = ep.tile([128, 1024], BF16, tag="e", name=tag("e"))
                for h in range(2):
                    hb = 64 * h
                    nc.tensor.matmul(
                        s_ps[:, 512 * h + off:512 * (h + 1)],
                        lhsT=kt_sb[hb:hb + 64,
                                   jb + 128 * kt:jb + 128 * (kt + 1)],
                        rhs=qt_sb[hb:hb + 64, qb + off:qb + QC],
                        start=True, stop=True)
                s3 = s_ps.rearrange("p (h c) -> p h c", c=512)
                e3 = e_sb.rearrange("p (h c) -> p h c", c=512)
                nc.scalar.activation(e3[:, :, off:512], s3[:, :, off:512],
                                     AF.Exp)
                if kt >= 4 * qc:
                    # diagonal block: zero strictly-lower (k > q).  In the
                    # final chunk the masks go to the Pool engine so the DVE
                    # is clear for the tail-critical normalize chain.
                    meng = nc.gpsimd if direct_norm else nc.vector
                    for h in range(2):
                        meng.tensor_mul(e3[:, h, off:off + 128],
                                        e3[:, h, off:off + 128],
                                        mk_sb[:, :])
                for h in range(2):
                    nc.tensor.matmul(
                        ctx[h][:, off:512],
                        lhsT=v_sb[:, 512 * kt + 128 * (2 * pair + h):
                                  512 * kt + 128 * (2 * pair + h + 1)],
                        rhs=e3[:, h, off:512],
                        start=(kt == 0), stop=(kt == kt_max))
                drain(ndrain)

            # normalize: copy ctx+den to SBUF (frees PSUM), then
            # ct[hb rows] = ctx * (1/den) with den broadcast to hb rows.
            # direct_norm (last q-chunk): read straight from psum (tiles are
            # never reused), phase-order the two heads' chains, and split
            # the muls by column so the first out-proj tiles unblock early.
            if direct_norm:
                rc = []
                for h in range(2):
                    hb = 64 * h
                    dr = 64 - hb
                    rcp = rp.tile([128, 512], F32, tag="r", name=tag(f"r{h}"))
                    rcb = rp.tile([128, 512], F32, tag="rb",
                                  name=tag(f"rb{h}"))
                    # write the reciprocal to partition 0 regardless of which
                    # rows hold the denominator, so the (fast) gpsimd
                    # partition broadcast serves both heads
                    nc.vector.reciprocal(rcp[0:1, :], ctx[h][dr:dr + 1, :])
                    nc.gpsimd.partition_broadcast(rcb[:, :], rcp[0:1, :])
                    rc.append(rcb)
                for a in (0, 256):
                    for h in range(2):
                        hb = 64 * h
                        nc.vector.tensor_mul(
                            ct_sb[hb:hb + 64, qb + a:qb + a + 256],
                            ctx[h][hb:hb + 64, a:a + 256],
                            rc[h][hb:hb + 64, a:a + 256])
                return
            for h in range(2):
                hb = 64 * h          # ctx rows for this head in its psum tile
                dr = 64 - hb         # denominator rows
                cd = cs.tile([128, 512], F32, tag="cd", name=tag(f"cd{h}"))
                nc.vector.tensor_copy(cd[:, :], ctx[h][:, :])
                rcp = rp.tile([128, 512], F32, tag="r", name=tag(f"r{h}"))
                rcb = rp.tile([128, 512], F32, tag="rb", name=tag(f"rb{h}"))
                nc.vector.reciprocal(rcp[dr:dr + 1, :], cd[dr:dr + 1, :])
                if dr == 0:
                    # gpsimd broadcast (reads true partition 0 only)
                    nc.gpsimd.partition_broadcast(rcb[:, :], rcp[0:1, :])
                else:
                    # issued from the Pool queue so a wait on the reciprocal
                    # can't head-of-line-block the SP queue's y writebacks
                    nc.gpsimd.dma_start(
                        out=rcb[hb:hb + 64, :],
                        in_=rcp[dr:dr + 1, :].unsqueeze(1)
                        .to_broadcast((1, 64, 512)))
                nc.vector.tensor_mul(
                    ct_sb[hb:hb + 64, qb:qb + QC],
                    cd[hb:hb + 64, :], rcb[hb:hb + 64, :])
                drain(ndrain)

        # ================= emission schedule =================
        # ramp the PE p-state while the first DMAs land
        warm_pe(14)
        # S1: minimum prefix for (pair 0, qc 0): K/Q j0 s0:512, V st0..3.
        run_now(qk_group(wk_sb, kt_sb, 0, 0, False, True))  # K j0 s0:512
        run_now(qk_group(wq_sb, qt_sb, 0, 0, False, True))  # Q j0 s0:512
        for st in range(4):
            run_now(v_group(st, True))

        # registry of remaining projection groups (also queued as fillers).
        kq_gen = {}
        for j in range(2):
            for sc in range(4):
                if j == 0 and sc == 0:
                    continue
                kq_gen[("k", j, sc)] = qk_group(wk_sb, kt_sb, j, sc,
                                                False, False)
                kq_gen[("q", j, sc)] = qk_group(wq_sb, qt_sb, j, sc,
                                                False, False)
        v_gen = {st: v_group(st, False) for st in range(4, 16)}

        # priority order: j1 s0:512 (pair-1 qc0), then the s512:1024
        # chunks (qc1), V st4..7 (qc1), then s1024:2048, then V st8..15.
        for key in (("k", 1, 0), ("q", 1, 0),
                    ("k", 0, 1), ("q", 0, 1), ("k", 1, 1), ("q", 1, 1)):
            fillers.append(kq_gen[key])
        for st in range(4, 8):
            fillers.append(v_gen[st])
        for sc in (2, 3):
            for j in range(2):
                fillers.append(kq_gen[("k", j, sc)])
                fillers.append(kq_gen[("q", j, sc)])
        for st in range(8, 16):
            fillers.append(v_gen[st])

        def need(pair, qc):
            """Force-emit everything attention(pair, qc) reads."""
            j = pair
            for sc in range(qc + 1):
                if (("k", j, sc)) in kq_gen:
                    run_now(kq_gen[("k", j, sc)])
            if ("q", j, qc) in kq_gen:
                run_now(kq_gen[("q", j, qc)])
            for st in range(4 * qc + 4):
                if st in v_gen:
                    run_now(v_gen[st])

        # fillers are drained during attention; two qc2 out-proj groups are
        # held out (ACT copies) to bridge the last normalize chains without
        # touching the DVE, keeping the PE busy and its p-state warm.
        late = []
        for qc in range(NQC):
            for pair in range(2):
                need(pair, qc)
                attention(pair, qc, ndrain=2,
                          direct_norm=(qc == NQC - 1 and pair == 1))
            if qc < NQC - 1:
                for st in range(4 * qc, 4 * qc + 4):
                    if qc == 2 and st >= 10:
                        late.append(oproj_st(st, True))
                    else:
                        fillers.append(oproj_st(st, False))

        # bridge the last normalize chains: keep the PE warm so the tail
        # matmuls run at full clock
        warm_pe(14)
        # tail: held-back groups first, then the last q-chunk's
        # out-projection on the now-free 's' psum ring.
        tail = deque(late)
        tail.extend(oproj_big(st, (st % 2 == 0), split=True)
                    for st in range(12, 16))
        while fillers or tail:
            drain(1)
            if tail:
                g = tail.popleft()
                try:
                    next(g)
                    tail.append(g)
                except StopIteration:
                    pass

    nc.compile()
    return nc


_nc = None


def make_in_maps(x, W_q, W_k, W_v, W_o):
    """Per-core input dict construction (shared with test.py)."""
    bf = ml_dtypes.bfloat16
    mask = np.triu(np.ones((128, 128), dtype=np.float32))  # 1 where k <= q
    in_maps = []
    for c in range(N_CORES):
        b = c // 4
        g = c % 4
        sl = slice(DC * g, DC * (g + 1))
        in_maps.append({
            "xt": np.ascontiguousarray(x[b].T).astype(bf),
            "wq": (np.ascontiguousarray(W_q[:, sl]) * 0.125).astype(bf),
            "wk": np.ascontiguousarray(W_k[:, sl]).astype(bf),
            "wv": np.ascontiguousarray(W_v[:, sl]).astype(bf),
            "wo": np.ascontiguousarray(W_o[sl, :]).astype(bf),
            "mk": mask.astype(bf),
        })
    return in_maps


def kernel(x, W_q, W_k, W_v, W_o, b_o):
    global _nc
    x = np.ascontiguousarray(np.asarray(x, dtype=np.float32))
    W_q = np.asarray(W_q, dtype=np.float32)
    W_k = np.asarray(W_k, dtype=np.float32)
    W_v = np.asarray(W_v, dtype=np.float32)
    W_o = np.asarray(W_o, dtype=np.float32)
    b_o = np.asarray(b_o, dtype=np.float32)

    if _nc is None:
        _nc = _build()

    in_maps = make_in_maps(x, W_q, W_k, W_v, W_o)
    res = run_bass_kernel_spmd(_nc, in_maps, list(range(N_CORES)))
    parts = [res.results[c]["y"] for c in range(N_CORES)]
    out = np.empty((B, S, D), dtype=np.float32)
    for b in range(B):
        acc = np.zeros((S, D), dtype=np.float64)
        for g in range(4):
            acc += np.asarray(parts[4 * b + g], dtype=np.float32)
        acc += b_o
        out[b] = acc.astype(np.float32)
    return out


# revision 49
# speedup vs baseline: 1.3251x; 1.0327x over previous
"""Multi-head causal attention (B=2, S=2048, D=1024, H=16, Dh=64) on 8 TRN2
NeuronCores.

Sharding: core c = 4*b + g handles batch b (2-way data parallel) and head
group g (4-way tensor parallel over the 16 heads: heads 4g..4g+3, i.e. a
256-column slice of W_q/W_k/W_v, and the matching 256-row slice of W_o).
Each core returns a partial output [S, D]; the host sums the 4 partials per
batch and adds b_o (row-parallel out-projection reduce).

On-core dataflow (all matmul operands bf16, accumulation fp32 in PSUM):
scores are computed transposed (S^T[k, q] = K q^T) so softmax's sum over k
is folded into the attn@V matmul by augmenting V's stationary tile with 64
columns of ones (denominator lands in the other half of the PSUM partition
range). Attention processes head PAIRS with 512-wide q-chunks: one score
PSUM tile holds both heads' scores for a k-tile so a single (strided)
activation computes exp for the pair.  The instruction stream is software-
pipelined: the exp-bound attention windows are back-filled with projection
and out-projection matmuls drained from a filler queue, so the PE stays
busy end to end.
"""

import numpy as np
from collections import deque
from contextlib import ExitStack

import ml_dtypes
import concourse.bass as bass
import concourse.bacc as bacc
import concourse.tile as tile
import concourse.mybir as mybir
from concourse.bass_utils import run_bass_kernel_spmd

F32 = mybir.dt.float32
BF16 = mybir.dt.bfloat16
AF = mybir.ActivationFunctionType

B = 2
S = 2048
D = 1024
DC = 256  # head dims per core (4 heads x 64)
N_CORES = 8
NT = D // 128  # 8 input-dim tiles
ST = S // 128  # 16 sequence tiles
QC = 512  # attention q-chunk
NQC = S // QC  # 4 q-chunks


def _build():
    nc = bacc.Bacc("TRN2", target_bir_lowering=False, debug=False,
                   num_devices=N_CORES)
    xt = nc.dram_tensor("xt", [D, S], BF16, kind="ExternalInput").ap()
    wq = nc.dram_tensor("wq", [D, DC], BF16, kind="ExternalInput").ap()
    wk = nc.dram_tensor("wk", [D, DC], BF16, kind="ExternalInput").ap()
    wv = nc.dram_tensor("wv", [D, DC], BF16, kind="ExternalInput").ap()
    wo = nc.dram_tensor("wo", [DC, D], BF16, kind="ExternalInput").ap()
    mk = nc.dram_tensor("mk", [128, 128], BF16, kind="ExternalInput").ap()
    y = nc.dram_tensor("y", [S, D], BF16, kind="ExternalOutput").ap()

    with tile.TileContext(nc) as tc, ExitStack() as stk:
        persist = stk.enter_context(tc.tile_pool(name="persist", bufs=1))
        # Q^T / K^T: j-block (heads 2j, 2j+1) at cols 2048j; head 2j on
        # partitions 0:64, head 2j+1 on 64:128.
        qt_sb = persist.tile([128, 2 * S], BF16)
        kt_sb = persist.tile([128, 2 * S], BF16)
        # V per k-tile block of 512 cols: head h sub-block of 128 cols =
        # [V_h | ones] for even h, [ones | V_h] for odd h.
        v_sb = persist.tile([128, ST * 512], BF16)
        ct_sb = persist.tile([128, 2 * S], BF16)   # normalized ctx^T
        wo_sb = persist.tile([128, 2 * D], BF16)   # W_o slice: d-tile at 1024d
        mk_sb = persist.tile([128, 128], BF16)     # mask[k, q] = (k <= q)
        xt_sb = persist.tile([128, NT * S], BF16)  # x^T: d-tile i at cols 2048i
        wq_sb = persist.tile([128, NT * DC], BF16)
        wk_sb = persist.tile([128, NT * DC], BF16)
        wv_sb = persist.tile([128, NT * DC], BF16)

        # ---- input DMAs (arrival order matters) ----
        def w_in(dst, src):
            nc.sync.dma_start(
                out=dst.rearrange("p (i c) -> p i c", c=DC),
                in_=src.rearrange("(i p) c -> p i c", p=128))

        xt3 = xt.rearrange("(i p) s -> p i s", p=128)
        xs3 = xt_sb.rearrange("p (i s) -> p i s", s=S)
        # minimum prefix for S1 first: W_k/W_q + x^T s-cols 0:512 (a full
        # weight costs the same DMA time as its j0 half: the half's 256-byte
        # runs pay the sub-512B 2x penalty)
        w_in(wk_sb, wk)
        for i0 in range(0, 8, 2):
            nc.sync.dma_start(out=xs3[:, i0:i0 + 2, 0:512],
                              in_=xt3[:, i0:i0 + 2, 0:512])
        w_in(wq_sb, wq)
        w_in(wv_sb, wv)
        nc.sync.dma_start(out=mk_sb[:], in_=mk[:, :])
        nc.sync.dma_start(out=xs3[:, :, 512:1024], in_=xt3[:, :, 512:1024])
        nc.sync.dma_start(out=xs3[:, :, 1024:2048], in_=xt3[:, :, 1024:2048])
        for d in range(2):
            nc.sync.dma_start(out=wo_sb[:, 1024 * d:1024 * (d + 1)],
                              in_=wo[128 * d:128 * (d + 1), :])

        # ones columns of v_sb (static): cols 64:192 of each 256 sub-block
        v3 = v_sb.rearrange("p (n c) -> p n c", c=256)
        scr_sb = persist.tile([128, 128], BF16)  # warm-up matmul scratch
        nc.vector.memset(scr_sb[:], 0.5)
        nc.vector.memset(v3[:, :, 64:192], 1.0)

        pp = stk.enter_context(tc.tile_pool(name="pp", bufs=2, space="PSUM"))
        sp = cp = op = pp
        wp = stk.enter_context(tc.tile_pool(name="wp", bufs=5))
        ep = rp = cs = ob = wp

        nid = [0]

        def tag(p):
            nid[0] += 1
            return f"{p}{nid[0]}"

        def warm_pe(n):
            """n throwaway matmuls on scratch data: keeps the PE busy (and
            its p-state ramp warm) across windows where real matmuls are
            blocked on DMA or on a cross-engine chain."""
            ps = sp.tile([128, 1024], F32, tag="s", name=tag("wm"))
            for _ in range(n):
                nc.tensor.matmul(ps[:, 0:128], lhsT=scr_sb[:, 0:128],
                                 rhs=scr_sb[:, 0:128], start=True, stop=True)

        # ---- projection group emitters (generators; yield = filler step) --
        def qk_group(w_sb, dst, j, sc, big, scalar_copy):
            """Q^T/K^T out rows = dq (j-block), s-cols [512sc, 512sc+w).
            big: [128,1024] psum on tag 's' (pre-attention, w=1024);
            else [128,512] on tag 'o' (filler, w=512)."""
            if big:
                ps = sp.tile([128, 1024], F32, tag="s", name=tag("pq"))
                cols = ((0, 512), (512, 1024))
            else:
                ps = op.tile([128, 512], F32, tag="o", name=tag("pq"))
                cols = ((0, 512),)
            for i in range(NT):
                for a, b in cols:
                    nc.tensor.matmul(
                        ps[:, a:b],
                        lhsT=w_sb[:, DC * i + 128 * j:DC * i + 128 * (j + 1)],
                        rhs=xt_sb[:, S * i + 512 * sc + a:
                                  S * i + 512 * sc + b],
                        start=(i == 0), stop=(i == NT - 1))
                if i % 4 == 3:
                    yield
            w = 1024 if big else 512
            dcol = 2048 * j + 512 * sc
            if scalar_copy:
                nc.scalar.activation(dst[:, dcol:dcol + w], ps[:, 0:w],
                                     AF.Copy)
            else:
                nc.vector.tensor_copy(dst[:, dcol:dcol + w], ps[:, 0:w])
            yield

        def v_group(st, scalar_copy):
            """V block st: out rows = s (128 of st), cols = 256 head dims."""
            ps = op.tile([128, 512], F32, tag="o", name=tag("pv"))
            for i in range(NT):
                nc.tensor.matmul(
                    ps[:, 0:256],
                    lhsT=xt_sb[:, S * i + 128 * st:S * i + 128 * (st + 1)],
                    rhs=wv_sb[:, DC * i:DC * (i + 1)],
                    start=(i == 0), stop=(i == NT - 1))
                if i % 4 == 3:
                    yield
            blk = v3[:, 2 * st:2 * st + 2, :]
            srcv = ps[:, 0:256].rearrange("p (h c) -> p h c", c=128)
            if scalar_copy:
                nc.scalar.activation(blk[:, :, 0:64], srcv[:, :, 0:64],
                                     AF.Copy)
                nc.scalar.activation(blk[:, :, 192:256], srcv[:, :, 64:128],
                                     AF.Copy)
            else:
                nc.vector.tensor_copy(blk[:, :, 0:64], srcv[:, :, 0:64])
                nc.vector.tensor_copy(blk[:, :, 192:256], srcv[:, :, 64:128])
            yield

        def oproj_group(st, half, scalar_copy, o_sb, tg="o"):
            """Out-projection for s-tile st, y-cols half*512."""
            ps = op.tile([128, 512], F32, tag=tg, name=tag("po"))
            for d in range(2):
                nc.tensor.matmul(
                    ps[:, 0:512],
                    lhsT=ct_sb[:, 2048 * d + 128 * st:
                               2048 * d + 128 * (st + 1)],
                    rhs=wo_sb[:, 1024 * d + 512 * half:
                              1024 * d + 512 * (half + 1)],
                    start=(d == 0), stop=(d == 1))
            yield
            if scalar_copy:
                nc.scalar.activation(o_sb[:, 512 * half:512 * (half + 1)],
                                     ps[:, 0:512], AF.Copy)
            else:
                nc.vector.tensor_copy(o_sb[:, 512 * half:512 * (half + 1)],
                                      ps[:, 0:512])
            yield

        def oproj_st(st, scalar_copy, tg="o"):
            o_sb = ob.tile([128, 1024], BF16, tag="ob", name=tag("ob"))
            yield from oproj_group(st, 0, scalar_copy, o_sb, tg)
            yield from oproj_group(st, 1, scalar_copy, o_sb, tg)
            nc.sync.dma_start(out=y[128 * st:128 * (st + 1), :], in_=o_sb[:])
            yield

        def oproj_big(st, scalar_copy, split=False):
            """Tail-era out-projection: full s-tile on the (now free) 's'
            psum ring.  split: halve the copy across ACT+DVE and the DMA
            across the SP+Pool queues to shorten the final drain."""
            ps = sp.tile([128, 1024], F32, tag="s", name=tag("pt"))
            o_sb = ob.tile([128, 1024], BF16, tag="ob", name=tag("obt"))
            for d in range(2):
                for a, b in ((0, 512), (512, 1024)):
                    nc.tensor.matmul(
                        ps[:, a:b],
                        lhsT=ct_sb[:, 2048 * d + 128 * st:
                                   2048 * d + 128 * (st + 1)],
                        rhs=wo_sb[:, 1024 * d + a:1024 * d + b],
                        start=(d == 0), stop=(d == 1))
            yield
            row = y[128 * st:128 * (st + 1), :]
            if split:
                nc.scalar.activation(o_sb[:, 0:512], ps[:, 0:512], AF.Copy)
                nc.vector.tensor_copy(o_sb[:, 512:1024], ps[:, 512:1024])
                yield
                nc.sync.dma_start(out=row[:, 0:512], in_=o_sb[:, 0:512])
                nc.sync.dma_start(out=row[:, 512:1024],
                                  in_=o_sb[:, 512:1024])
                yield
                return
            if scalar_copy:
                nc.scalar.activation(o_sb[:], ps[:], AF.Copy)
            else:
                nc.vector.tensor_copy(o_sb[:], ps[:])
            yield
            nc.sync.dma_start(out=row[:], in_=o_sb[:])
            yield

        fillers = deque()
        reserve = [0]  # generators held back for the post-attention window

        def drain(n=1):
            for _ in range(n):
                if len(fillers) <= reserve[0]:
                    return
                advanced = False
                while len(fillers) > reserve[0] and not advanced:
                    try:
                        next(fillers[0])
                        advanced = True
                    except StopIteration:
                        fillers.popleft()
                if not advanced:
                    return

        def run_now(gen):
            for _ in gen:
                pass

        # ---- attention for a head pair on one q-chunk ----
        def attention(pair, qc, ndrain=1, direct_norm=False):
            """pair: 0 -> heads 0,1 (j-block 0); 1 -> heads 2,3 (j-block 1).
            q-chunk = [QC*qc, QC*(qc+1))."""
            jb = 2048 * pair
            qb = jb + QC * qc
            kt_max = 4 * qc + 3
            ctx = [cp.tile([128, 512], F32, tag="c", name=tag(f"cx{h}"))
                   for h in range(2)]
            for kt in range(kt_max + 1):
                off = max(0, 128 * kt - QC * qc)
                s_ps = sp.tile([128, 1024], F32, tag="s", name=tag("s"))
                e_sb = ep.tile([128, 1024], BF16, tag="e", bufs=8, name=tag("e"))
                for h in range(2):
                    hb = 64 * h
                    nc.tensor.matmul(
                        s_ps[:, 512 * h + off:512 * (h + 1)],
                        lhsT=kt_sb[hb:hb + 64,
                                   jb + 128 * kt:jb + 128 * (kt + 1)],
                        rhs=qt_sb[hb:hb + 64, qb + off:qb + QC],
                        start=True, stop=True)
                s3 = s_ps.rearrange("p (h c) -> p h c", c=512)
                e3 = e_sb.rearrange("p (h c) -> p h c", c=512)
                nc.scalar.activation(e3[:, :, off:512], s3[:, :, off:512],
                                     AF.Exp)
                if kt >= 4 * qc:
                    # diagonal block: zero strictly-lower (k > q).  In the
                    # final chunk the masks go to the Pool engine so the DVE
                    # is clear for the tail-critical normalize chain.
                    meng = nc.gpsimd if direct_norm else nc.vector
                    for h in range(2):
                        meng.tensor_mul(e3[:, h, off:off + 128],
                                        e3[:, h, off:off + 128],
                                        mk_sb[:, :])
                for h in range(2):
                    nc.tensor.matmul(
                        ctx[h][:, off:512],
                        lhsT=v_sb[:, 512 * kt + 128 * (2 * pair + h):
                                  512 * kt + 128 * (2 * pair + h + 1)],
                        rhs=e3[:, h, off:512],
                        start=(kt == 0), stop=(kt == kt_max))
                drain(ndrain)

            # normalize: copy ctx+den to SBUF (frees PSUM), then
            # ct[hb rows] = ctx * (1/den) with den broadcast to hb rows.
            # direct_norm (last q-chunk): read straight from psum (tiles are
            # never reused), phase-order the two heads' chains, and split
            # the muls by column so the first out-proj tiles unblock early.
            if direct_norm:
                rc = []
                for h in range(2):
                    hb = 64 * h
                    dr = 64 - hb
                    rcp = rp.tile([128, 512], F32, tag="r", name=tag(f"r{h}"))
                    rcb = rp.tile([128, 512], F32, tag="rb",
                                  name=tag(f"rb{h}"))
                    # write the reciprocal to partition 0 regardless of which
                    # rows hold the denominator, so the (fast) gpsimd
                    # partition broadcast serves both heads
                    nc.vector.reciprocal(rcp[0:1, :], ctx[h][dr:dr + 1, :])
                    nc.gpsimd.partition_broadcast(rcb[:, :], rcp[0:1, :])
                    rc.append(rcb)
                for a in (0, 256):
                    for h in range(2):
                        hb = 64 * h
                        nc.vector.tensor_mul(
                            ct_sb[hb:hb + 64, qb + a:qb + a + 256],
                            ctx[h][hb:hb + 64, a:a + 256],
                            rc[h][hb:hb + 64, a:a + 256])
                return
            for h in range(2):
                hb = 64 * h          # ctx rows for this head in its psum tile
                dr = 64 - hb         # denominator rows
                cd = cs.tile([128, 512], F32, tag="cd", name=tag(f"cd{h}"))
                nc.vector.tensor_copy(cd[:, :], ctx[h][:, :])
                rcp = rp.tile([128, 512], F32, tag="r", name=tag(f"r{h}"))
                rcb = rp.tile([128, 512], F32, tag="rb", name=tag(f"rb{h}"))
                nc.vector.reciprocal(rcp[dr:dr + 1, :], cd[dr:dr + 1, :])
                if dr == 0:
                    # gpsimd broadcast (reads true partition 0 only)
                    nc.gpsimd.partition_broadcast(rcb[:, :], rcp[0:1, :])
                else:
                    # issued from the Pool queue so a wait on the reciprocal
                    # can't head-of-line-block the SP queue's y writebacks
                    nc.gpsimd.dma_start(
                        out=rcb[hb:hb + 64, :],
                        in_=rcp[dr:dr + 1, :].unsqueeze(1)
                        .to_broadcast((1, 64, 512)))
                nc.vector.tensor_mul(
                    ct_sb[hb:hb + 64, qb:qb + QC],
                    cd[hb:hb + 64, :], rcb[hb:hb + 64, :])
                drain(ndrain)

        # ================= emission schedule =================
        # ramp the PE p-state while the first DMAs land
        warm_pe(22)
        # S1: minimum prefix for (pair 0, qc 0): K/Q j0 s0:512, V st0..3.
        run_now(qk_group(wk_sb, kt_sb, 0, 0, False, True))  # K j0 s0:512
        run_now(qk_group(wq_sb, qt_sb, 0, 0, False, True))  # Q j0 s0:512
        for st in range(4):
            run_now(v_group(st, True))

        # registry of remaining projection groups (also queued as fillers).
        kq_gen = {}
        for j in range(2):
            for sc in range(4):
                if j == 0 and sc == 0:
                    continue
                kq_gen[("k", j, sc)] = qk_group(wk_sb, kt_sb, j, sc,
                                                False, False)
                kq_gen[("q", j, sc)] = qk_group(wq_sb, qt_sb, j, sc,
                                                False, False)
        v_gen = {st: v_group(st, False) for st in range(4, 16)}

        # priority order: j1 s0:512 (pair-1 qc0), then the s512:1024
        # chunks (qc1), V st4..7 (qc1), then s1024:2048, then V st8..15.
        for key in (("k", 1, 0), ("q", 1, 0),
                    ("k", 0, 1), ("q", 0, 1), ("k", 1, 1), ("q", 1, 1)):
            fillers.append(kq_gen[key])
        for st in range(4, 8):
            fillers.append(v_gen[st])
        for sc in (2, 3):
            for j in range(2):
                fillers.append(kq_gen[("k", j, sc)])
                fillers.append(kq_gen[("q", j, sc)])
        for st in range(8, 16):
            fillers.append(v_gen[st])

        def need(pair, qc):
            """Force-emit everything attention(pair, qc) reads."""
            j = pair
            for sc in range(qc + 1):
                if (("k", j, sc)) in kq_gen:
                    run_now(kq_gen[("k", j, sc)])
            if ("q", j, qc) in kq_gen:
                run_now(kq_gen[("q", j, qc)])
            for st in range(4 * qc + 4):
                if st in v_gen:
                    run_now(v_gen[st])

        # fillers are drained during attention; two qc2 out-proj groups are
        # held out (ACT copies) to bridge the last normalize chains without
        # touching the DVE, keeping the PE busy and its p-state warm.
        late = []
        for qc in range(NQC):
            # during qc1, shield the (just queued) qc0 out-proj groups so
            # they land in the filler-starved qc2/qc3 windows instead
            reserve[0] = 4 if qc == 1 else 0
            for pair in range(2):
                need(pair, qc)
                attention(pair, qc, ndrain=(2 if qc < 2 else 3),
                          direct_norm=(qc == NQC - 1 and pair == 1))
            if qc < NQC - 1:
                for st in range(4 * qc, 4 * qc + 4):
                    if qc == 2 and st >= 10:
                        late.append(oproj_st(st, True))
                    else:
                        fillers.append(oproj_st(st, False))

        # bridge the last normalize chains: keep the PE warm so the tail
        # matmuls run at full clock
        warm_pe(20)
        # tail: held-back groups first, then the last q-chunk's
        # out-projection on the now-free 's' psum ring.
        tail = deque(late)
        tail.append(oproj_big(12, True))
        tail.append(oproj_st(13, True, "o"))
        tail.append(oproj_st(14, False, "c"))
        tail.append(oproj_big(15, False, split=True))
        while fillers or tail:
            drain(1)
            if tail:
                g = tail.popleft()
                try:
                    next(g)
                    tail.append(g)
                except StopIteration:
                    pass

    nc.compile()
    return nc


_nc = None


def make_in_maps(x, W_q, W_k, W_v, W_o):
    """Per-core input dict construction (shared with test.py)."""
    bf = ml_dtypes.bfloat16
    mask = np.triu(np.ones((128, 128), dtype=np.float32))  # 1 where k <= q
    in_maps = []
    for c in range(N_CORES):
        b = c // 4
        g = c % 4
        sl = slice(DC * g, DC * (g + 1))
        in_maps.append({
            "xt": np.ascontiguousarray(x[b].T).astype(bf),
            "wq": (np.ascontiguousarray(W_q[:, sl]) * 0.125).astype(bf),
            "wk": np.ascontiguousarray(W_k[:, sl]).astype(bf),
            "wv": np.ascontiguousarray(W_v[:, sl]).astype(bf),
            "wo": np.ascontiguousarray(W_o[sl, :]).astype(bf),
            "mk": mask.astype(bf),
        })
    return in_maps


def kernel(x, W_q, W_k, W_v, W_o, b_o):
    global _nc
    x = np.ascontiguousarray(np.asarray(x, dtype=np.float32))
    W_q = np.asarray(W_q, dtype=np.float32)
    W_k = np.asarray(W_k, dtype=np.float32)
    W_v = np.asarray(W_v, dtype=np.float32)
    W_o = np.asarray(W_o, dtype=np.float32)
    b_o = np.asarray(b_o, dtype=np.float32)

    if _nc is None:
        _nc = _build()

    in_maps = make_in_maps(x, W_q, W_k, W_v, W_o)
    res = run_bass_kernel_spmd(_nc, in_maps, list(range(N_CORES)))
    parts = [res.results[c]["y"] for c in range(N_CORES)]
    out = np.empty((B, S, D), dtype=np.float32)
    for b in range(B):
        acc = np.zeros((S, D), dtype=np.float64)
        for g in range(4):
            acc += np.asarray(parts[4 * b + g], dtype=np.float32)
        acc += b_o
        out[b] = acc.astype(np.float32)
    return out


# revision 50
# speedup vs baseline: 1.3316x; 1.0049x over previous
"""Multi-head causal attention (B=2, S=2048, D=1024, H=16, Dh=64) on 8 TRN2
NeuronCores.

Sharding: core c = 4*b + g handles batch b (2-way data parallel) and head
group g (4-way tensor parallel over the 16 heads: heads 4g..4g+3, i.e. a
256-column slice of W_q/W_k/W_v, and the matching 256-row slice of W_o).
Each core returns a partial output [S, D]; the host sums the 4 partials per
batch and adds b_o (row-parallel out-projection reduce).

On-core dataflow (all matmul operands bf16, accumulation fp32 in PSUM):
scores are computed transposed (S^T[k, q] = K q^T) so softmax's sum over k
is folded into the attn@V matmul by augmenting V's stationary tile with 64
columns of ones (denominator lands in the other half of the PSUM partition
range). Attention processes head PAIRS with 512-wide q-chunks: one score
PSUM tile holds both heads' scores for a k-tile so a single (strided)
activation computes exp for the pair.  The instruction stream is software-
pipelined: the exp-bound attention windows are back-filled with projection
and out-projection matmuls drained from a filler queue, so the PE stays
busy end to end.
"""

import numpy as np
from collections import deque
from contextlib import ExitStack

import ml_dtypes
import concourse.bass as bass
import concourse.bacc as bacc
import concourse.tile as tile
import concourse.mybir as mybir
from concourse.bass_utils import run_bass_kernel_spmd

F32 = mybir.dt.float32
BF16 = mybir.dt.bfloat16
AF = mybir.ActivationFunctionType

B = 2
S = 2048
D = 1024
DC = 256  # head dims per core (4 heads x 64)
N_CORES = 8
NT = D // 128  # 8 input-dim tiles
ST = S // 128  # 16 sequence tiles
QC = 512  # attention q-chunk
NQC = S // QC  # 4 q-chunks


def _build():
    nc = bacc.Bacc("TRN2", target_bir_lowering=False, debug=False,
                   num_devices=N_CORES)
    xt = nc.dram_tensor("xt", [D, S], BF16, kind="ExternalInput").ap()
    wq = nc.dram_tensor("wq", [D, DC], BF16, kind="ExternalInput").ap()
    wk = nc.dram_tensor("wk", [D, DC], BF16, kind="ExternalInput").ap()
    wv = nc.dram_tensor("wv", [D, DC], BF16, kind="ExternalInput").ap()
    wo = nc.dram_tensor("wo", [DC, D], BF16, kind="ExternalInput").ap()
    mk = nc.dram_tensor("mk", [128, 128], BF16, kind="ExternalInput").ap()
    y = nc.dram_tensor("y", [S, D], BF16, kind="ExternalOutput").ap()

    with tile.TileContext(nc) as tc, ExitStack() as stk:
        persist = stk.enter_context(tc.tile_pool(name="persist", bufs=1))
        # Q^T / K^T: j-block (heads 2j, 2j+1) at cols 2048j; head 2j on
        # partitions 0:64, head 2j+1 on 64:128.
        qt_sb = persist.tile([128, 2 * S], BF16)
        kt_sb = persist.tile([128, 2 * S], BF16)
        # V per k-tile block of 512 cols: head h sub-block of 128 cols =
        # [V_h | ones] for even h, [ones | V_h] for odd h.
        v_sb = persist.tile([128, ST * 512], BF16)
        ct_sb = persist.tile([128, 2 * S], BF16)   # normalized ctx^T
        wo_sb = persist.tile([128, 2 * D], BF16)   # W_o slice: d-tile at 1024d
        mk_sb = persist.tile([128, 128], BF16)     # mask[k, q] = (k <= q)
        xt_sb = persist.tile([128, NT * S], BF16)  # x^T: d-tile i at cols 2048i
        wq_sb = persist.tile([128, NT * DC], BF16)
        wk_sb = persist.tile([128, NT * DC], BF16)
        wv_sb = persist.tile([128, NT * DC], BF16)

        # ---- input DMAs (arrival order matters) ----
        def w_in(dst, src):
            nc.sync.dma_start(
                out=dst.rearrange("p (i c) -> p i c", c=DC),
                in_=src.rearrange("(i p) c -> p i c", p=128))

        xt3 = xt.rearrange("(i p) s -> p i s", p=128)
        xs3 = xt_sb.rearrange("p (i s) -> p i s", s=S)
        # minimum prefix for S1 first: W_k/W_q + x^T s-cols 0:512 (a full
        # weight costs the same DMA time as its j0 half: the half's 256-byte
        # runs pay the sub-512B 2x penalty)
        w_in(wk_sb, wk)
        for i0 in range(0, 8, 2):
            nc.sync.dma_start(out=xs3[:, i0:i0 + 2, 0:512],
                              in_=xt3[:, i0:i0 + 2, 0:512])
        w_in(wq_sb, wq)
        w_in(wv_sb, wv)
        nc.sync.dma_start(out=mk_sb[:], in_=mk[:, :])
        nc.sync.dma_start(out=xs3[:, :, 512:1024], in_=xt3[:, :, 512:1024])
        nc.sync.dma_start(out=xs3[:, :, 1024:2048], in_=xt3[:, :, 1024:2048])
        for d in range(2):
            nc.sync.dma_start(out=wo_sb[:, 1024 * d:1024 * (d + 1)],
                              in_=wo[128 * d:128 * (d + 1), :])

        # ones columns of v_sb (static): cols 64:192 of each 256 sub-block
        v3 = v_sb.rearrange("p (n c) -> p n c", c=256)
        scr_sb = persist.tile([128, 128], BF16)  # warm-up matmul scratch
        nc.vector.memset(scr_sb[:], 0.5)
        nc.vector.memset(v3[:, :, 64:192], 1.0)

        pp = stk.enter_context(tc.tile_pool(name="pp", bufs=2, space="PSUM"))
        sp = cp = op = pp
        wp = stk.enter_context(tc.tile_pool(name="wp", bufs=5))
        ep = rp = cs = ob = wp

        nid = [0]

        def tag(p):
            nid[0] += 1
            return f"{p}{nid[0]}"

        def warm_pe(n):
            """n throwaway matmuls on scratch data: keeps the PE busy (and
            its p-state ramp warm) across windows where real matmuls are
            blocked on DMA or on a cross-engine chain."""
            ps = sp.tile([128, 1024], F32, tag="s", name=tag("wm"))
            for _ in range(n):
                nc.tensor.matmul(ps[:, 0:128], lhsT=scr_sb[:, 0:128],
                                 rhs=scr_sb[:, 0:128], start=True, stop=True)

        # ---- projection group emitters (generators; yield = filler step) --
        def qk_group(w_sb, dst, j, sc, big, scalar_copy):
            """Q^T/K^T out rows = dq (j-block), s-cols [512sc, 512sc+w).
            big: [128,1024] psum on tag 's' (pre-attention, w=1024);
            else [128,512] on tag 'o' (filler, w=512)."""
            if big:
                ps = sp.tile([128, 1024], F32, tag="s", name=tag("pq"))
                cols = ((0, 512), (512, 1024))
            else:
                ps = op.tile([128, 512], F32, tag="o", name=tag("pq"))
                cols = ((0, 512),)
            for i in range(NT):
                for a, b in cols:
                    nc.tensor.matmul(
                        ps[:, a:b],
                        lhsT=w_sb[:, DC * i + 128 * j:DC * i + 128 * (j + 1)],
                        rhs=xt_sb[:, S * i + 512 * sc + a:
                                  S * i + 512 * sc + b],
                        start=(i == 0), stop=(i == NT - 1))
                if i % 4 == 3:
                    yield
            w = 1024 if big else 512
            dcol = 2048 * j + 512 * sc
            if scalar_copy:
                nc.scalar.activation(dst[:, dcol:dcol + w], ps[:, 0:w],
                                     AF.Copy)
            else:
                nc.vector.tensor_copy(dst[:, dcol:dcol + w], ps[:, 0:w])
            yield

        def v_group(st, scalar_copy):
            """V block st: out rows = s (128 of st), cols = 256 head dims."""
            ps = op.tile([128, 512], F32, tag="o", name=tag("pv"))
            for i in range(NT):
                nc.tensor.matmul(
                    ps[:, 0:256],
                    lhsT=xt_sb[:, S * i + 128 * st:S * i + 128 * (st + 1)],
                    rhs=wv_sb[:, DC * i:DC * (i + 1)],
                    start=(i == 0), stop=(i == NT - 1))
                if i % 4 == 3:
                    yield
            blk = v3[:, 2 * st:2 * st + 2, :]
            srcv = ps[:, 0:256].rearrange("p (h c) -> p h c", c=128)
            if scalar_copy:
                nc.scalar.activation(blk[:, :, 0:64], srcv[:, :, 0:64],
                                     AF.Copy)
                nc.scalar.activation(blk[:, :, 192:256], srcv[:, :, 64:128],
                                     AF.Copy)
            else:
                nc.vector.tensor_copy(blk[:, :, 0:64], srcv[:, :, 0:64])
                nc.vector.tensor_copy(blk[:, :, 192:256], srcv[:, :, 64:128])
            yield

        def oproj_group(st, half, scalar_copy, o_sb, tg="o"):
            """Out-projection for s-tile st, y-cols half*512."""
            ps = op.tile([128, 512], F32, tag=tg, name=tag("po"))
            for d in range(2):
                nc.tensor.matmul(
                    ps[:, 0:512],
                    lhsT=ct_sb[:, 2048 * d + 128 * st:
                               2048 * d + 128 * (st + 1)],
                    rhs=wo_sb[:, 1024 * d + 512 * half:
                              1024 * d + 512 * (half + 1)],
                    start=(d == 0), stop=(d == 1))
            yield
            if scalar_copy:
                nc.scalar.activation(o_sb[:, 512 * half:512 * (half + 1)],
                                     ps[:, 0:512], AF.Copy)
            else:
                nc.vector.tensor_copy(o_sb[:, 512 * half:512 * (half + 1)],
                                      ps[:, 0:512])
            yield

        def oproj_st(st, scalar_copy, tg="o"):
            o_sb = ob.tile([128, 1024], BF16, tag="ob", name=tag("ob"))
            yield from oproj_group(st, 0, scalar_copy, o_sb, tg)
            yield from oproj_group(st, 1, scalar_copy, o_sb, tg)
            nc.sync.dma_start(out=y[128 * st:128 * (st + 1), :], in_=o_sb[:])
            yield

        def oproj_big(st, scalar_copy, split=False):
            """Tail-era out-projection: full s-tile on the (now free) 's'
            psum ring.  split: halve the copy across ACT+DVE and the DMA
            across the SP+Pool queues to shorten the final drain."""
            ps = sp.tile([128, 1024], F32, tag="s", name=tag("pt"))
            o_sb = ob.tile([128, 1024], BF16, tag="ob", name=tag("obt"))
            for d in range(2):
                for a, b in ((0, 512), (512, 1024)):
                    nc.tensor.matmul(
                        ps[:, a:b],
                        lhsT=ct_sb[:, 2048 * d + 128 * st:
                                   2048 * d + 128 * (st + 1)],
                        rhs=wo_sb[:, 1024 * d + a:1024 * d + b],
                        start=(d == 0), stop=(d == 1))
            yield
            row = y[128 * st:128 * (st + 1), :]
            if split:
                nc.scalar.activation(o_sb[:, 0:512], ps[:, 0:512], AF.Copy)
                nc.vector.tensor_copy(o_sb[:, 512:1024], ps[:, 512:1024])
                yield
                nc.sync.dma_start(out=row[:, 0:512], in_=o_sb[:, 0:512])
                nc.sync.dma_start(out=row[:, 512:1024],
                                  in_=o_sb[:, 512:1024])
                yield
                return
            if scalar_copy:
                nc.scalar.activation(o_sb[:], ps[:], AF.Copy)
            else:
                nc.vector.tensor_copy(o_sb[:], ps[:])
            yield
            nc.sync.dma_start(out=row[:], in_=o_sb[:])
            yield

        fillers = deque()
        reserve = [0]  # generators held back for the post-attention window

        def drain(n=1):
            for _ in range(n):
                if len(fillers) <= reserve[0]:
                    return
                advanced = False
                while len(fillers) > reserve[0] and not advanced:
                    try:
                        next(fillers[0])
                        advanced = True
                    except StopIteration:
                        fillers.popleft()
                if not advanced:
                    return

        def run_now(gen):
            for _ in gen:
                pass

        # ---- attention for a head pair on one q-chunk ----
        def attention(pair, qc, ndrain=1, direct_norm=False):
            """pair: 0 -> heads 0,1 (j-block 0); 1 -> heads 2,3 (j-block 1).
            q-chunk = [QC*qc, QC*(qc+1))."""
            jb = 2048 * pair
            qb = jb + QC * qc
            kt_max = 4 * qc + 3
            ctx = [cp.tile([128, 512], F32, tag="c", name=tag(f"cx{h}"))
                   for h in range(2)]
            for kt in range(kt_max + 1):
                off = max(0, 128 * kt - QC * qc)
                s_ps = sp.tile([128, 1024], F32, tag="s", name=tag("s"))
                e_sb = ep.tile([128, 1024], BF16, tag="e", bufs=8, name=tag("e"))
                for h in range(2):
                    hb = 64 * h
                    nc.tensor.matmul(
                        s_ps[:, 512 * h + off:512 * (h + 1)],
                        lhsT=kt_sb[hb:hb + 64,
                                   jb + 128 * kt:jb + 128 * (kt + 1)],
                        rhs=qt_sb[hb:hb + 64, qb + off:qb + QC],
                        start=True, stop=True)
                s3 = s_ps.rearrange("p (h c) -> p h c", c=512)
                e3 = e_sb.rearrange("p (h c) -> p h c", c=512)
                nc.scalar.activation(e3[:, :, off:512], s3[:, :, off:512],
                                     AF.Exp)
                if kt >= 4 * qc:
                    # diagonal block: zero strictly-lower (k > q).  In the
                    # final chunk the masks go to the Pool engine so the DVE
                    # is clear for the tail-critical normalize chain.
                    meng = nc.gpsimd if direct_norm else nc.vector
                    for h in range(2):
                        meng.tensor_mul(e3[:, h, off:off + 128],
                                        e3[:, h, off:off + 128],
                                        mk_sb[:, :])
                for h in range(2):
                    nc.tensor.matmul(
                        ctx[h][:, off:512],
                        lhsT=v_sb[:, 512 * kt + 128 * (2 * pair + h):
                                  512 * kt + 128 * (2 * pair + h + 1)],
                        rhs=e3[:, h, off:512],
                        start=(kt == 0), stop=(kt == kt_max))
                drain(ndrain)

            # normalize: copy ctx+den to SBUF (frees PSUM), then
            # ct[hb rows] = ctx * (1/den) with den broadcast to hb rows.
            # direct_norm (last q-chunk): read straight from psum (tiles are
            # never reused), phase-order the two heads' chains, and split
            # the muls by column so the first out-proj tiles unblock early.
            if direct_norm:
                rc = []
                for h in range(2):
                    hb = 64 * h
                    dr = 64 - hb
                    rcp = rp.tile([128, 512], F32, tag="r", name=tag(f"r{h}"))
                    rcb = rp.tile([128, 512], F32, tag="rb",
                                  name=tag(f"rb{h}"))
                    # write the reciprocal to partition 0 regardless of which
                    # rows hold the denominator, so the (fast) gpsimd
                    # partition broadcast serves both heads
                    nc.vector.reciprocal(rcp[0:1, :], ctx[h][dr:dr + 1, :])
                    nc.gpsimd.partition_broadcast(rcb[:, :], rcp[0:1, :])
                    rc.append(rcb)
                for a in (0, 256):
                    for h in range(2):
                        hb = 64 * h
                        nc.vector.tensor_mul(
                            ct_sb[hb:hb + 64, qb + a:qb + a + 256],
                            ctx[h][hb:hb + 64, a:a + 256],
                            rc[h][hb:hb + 64, a:a + 256])
                return
            for h in range(2):
                hb = 64 * h          # ctx rows for this head in its psum tile
                dr = 64 - hb         # denominator rows
                cd = cs.tile([128, 512], F32, tag="cd", name=tag(f"cd{h}"))
                nc.vector.tensor_copy(cd[:, :], ctx[h][:, :])
                rcp = rp.tile([128, 512], F32, tag="r", name=tag(f"r{h}"))
                rcb = rp.tile([128, 512], F32, tag="rb", name=tag(f"rb{h}"))
                nc.vector.reciprocal(rcp[dr:dr + 1, :], cd[dr:dr + 1, :])
                if dr == 0:
                    # gpsimd broadcast (reads true partition 0 only)
                    nc.gpsimd.partition_broadcast(rcb[:, :], rcp[0:1, :])
                else:
                    # issued from the Pool queue so a wait on the reciprocal
                    # can't head-of-line-block the SP queue's y writebacks
                    nc.gpsimd.dma_start(
                        out=rcb[hb:hb + 64, :],
                        in_=rcp[dr:dr + 1, :].unsqueeze(1)
                        .to_broadcast((1, 64, 512)))
                nc.vector.tensor_mul(
                    ct_sb[hb:hb + 64, qb:qb + QC],
                    cd[hb:hb + 64, :], rcb[hb:hb + 64, :])
                drain(ndrain)

        # ================= emission schedule =================
        # ramp the PE p-state while the first DMAs land
        warm_pe(22)
        # S1: minimum prefix for (pair 0, qc 0): K/Q j0 s0:512, V st0..3.
        run_now(qk_group(wk_sb, kt_sb, 0, 0, False, True))  # K j0 s0:512
        run_now(qk_group(wq_sb, qt_sb, 0, 0, False, True))  # Q j0 s0:512
        for st in range(4):
            run_now(v_group(st, True))

        # registry of remaining projection groups (also queued as fillers).
        kq_gen = {}
        for j in range(2):
            for sc in range(4):
                if j == 0 and sc == 0:
                    continue
                kq_gen[("k", j, sc)] = qk_group(wk_sb, kt_sb, j, sc,
                                                False, False)
                kq_gen[("q", j, sc)] = qk_group(wq_sb, qt_sb, j, sc,
                                                False, False)
        v_gen = {st: v_group(st, False) for st in range(4, 16)}

        # priority order: j1 s0:512 (pair-1 qc0), then the s512:1024
        # chunks (qc1), V st4..7 (qc1), then s1024:2048, then V st8..15.
        for key in (("k", 1, 0), ("q", 1, 0),
                    ("k", 0, 1), ("q", 0, 1), ("k", 1, 1), ("q", 1, 1)):
            fillers.append(kq_gen[key])
        for st in range(4, 8):
            fillers.append(v_gen[st])
        for sc in (2, 3):
            for j in range(2):
                fillers.append(kq_gen[("k", j, sc)])
                fillers.append(kq_gen[("q", j, sc)])
        for st in range(8, 16):
            fillers.append(v_gen[st])

        def need(pair, qc):
            """Force-emit everything attention(pair, qc) reads."""
            j = pair
            for sc in range(qc + 1):
                if (("k", j, sc)) in kq_gen:
                    run_now(kq_gen[("k", j, sc)])
            if ("q", j, qc) in kq_gen:
                run_now(kq_gen[("q", j, qc)])
            for st in range(4 * qc + 4):
                if st in v_gen:
                    run_now(v_gen[st])

        # fillers are drained during attention; two qc2 out-proj groups are
        # held out (ACT copies) to bridge the last normalize chains without
        # touching the DVE, keeping the PE busy and its p-state warm.
        late = []
        for qc in range(NQC):
            # during qc1, shield the (just queued) qc0 out-proj groups so
            # they land in the filler-starved qc2/qc3 windows instead
            reserve[0] = 4 if qc == 1 else 0
            for pair in range(2):
                need(pair, qc)
                attention(pair, qc, ndrain=(2 if qc < 2 else 3),
                          direct_norm=(qc == NQC - 1 and pair == 1))
            if qc < NQC - 1:
                for st in range(4 * qc, 4 * qc + 4):
                    if qc == 2 and st >= 10:
                        late.append(oproj_st(st, True))
                    else:
                        fillers.append(oproj_st(st, False))

        # bridge the last normalize chains: keep the PE warm so the tail
        # matmuls run at full clock
        warm_pe(20)
        # tail: held-back groups first, then the last q-chunk's
        # out-projection on the now-free 's' psum ring.
        tail = deque(late)
        tail.append(oproj_st(12, True, "o"))
        tail.append(oproj_big(13, True))
        tail.append(oproj_big(14, False))
        tail.append(oproj_st(15, False, "c"))
        while fillers or tail:
            drain(1)
            if tail:
                g = tail.popleft()
                try:
                    next(g)
                    tail.append(g)
                except StopIteration:
                    pass

    nc.compile()
    return nc


_nc = None


def make_in_maps(x, W_q, W_k, W_v, W_o):
    """Per-core input dict construction (shared with test.py)."""
    bf = ml_dtypes.bfloat16
    mask = np.triu(np.ones((128, 128), dtype=np.float32))  # 1 where k <= q
    in_maps = []
    for c in range(N_CORES):
        b = c // 4
        g = c % 4
        sl = slice(DC * g, DC * (g + 1))
        in_maps.append({
            "xt": np.ascontiguousarray(x[b].T).astype(bf),
            "wq": (np.ascontiguousarray(W_q[:, sl]) * 0.125).astype(bf),
            "wk": np.ascontiguousarray(W_k[:, sl]).astype(bf),
            "wv": np.ascontiguousarray(W_v[:, sl]).astype(bf),
            "wo": np.ascontiguousarray(W_o[sl, :]).astype(bf),
            "mk": mask.astype(bf),
        })
    return in_maps


def kernel(x, W_q, W_k, W_v, W_o, b_o):
    global _nc
    x = np.ascontiguousarray(np.asarray(x, dtype=np.float32))
    W_q = np.asarray(W_q, dtype=np.float32)
    W_k = np.asarray(W_k, dtype=np.float32)
    W_v = np.asarray(W_v, dtype=np.float32)
    W_o = np.asarray(W_o, dtype=np.float32)
    b_o = np.asarray(b_o, dtype=np.float32)

    if _nc is None:
        _nc = _build()

    in_maps = make_in_maps(x, W_q, W_k, W_v, W_o)
    res = run_bass_kernel_spmd(_nc, in_maps, list(range(N_CORES)))
    parts = [res.results[c]["y"] for c in range(N_CORES)]
    out = np.empty((B, S, D), dtype=np.float32)
    for b in range(B):
        acc = np.zeros((S, D), dtype=np.float64)
        for g in range(4):
            acc += np.asarray(parts[4 * b + g], dtype=np.float32)
        acc += b_o
        out[b] = acc.astype(np.float32)
    return out


# revision 55
# speedup vs baseline: 1.3403x; 1.0065x over previous
"""Multi-head causal attention (B=2, S=2048, D=1024, H=16, Dh=64) on 8 TRN2
NeuronCores.

Sharding: core c = 4*b + g handles batch b (2-way data parallel) and head
group g (4-way tensor parallel over the 16 heads: heads 4g..4g+3, i.e. a
256-column slice of W_q/W_k/W_v, and the matching 256-row slice of W_o).
Each core returns a partial output [S, D]; the host sums the 4 partials per
batch and adds b_o (row-parallel out-projection reduce).

On-core dataflow (all matmul operands bf16, accumulation fp32 in PSUM):
scores are computed transposed (S^T[k, q] = K q^T) so softmax's sum over k
is folded into the attn@V matmul by augmenting V's stationary tile with 64
columns of ones (denominator lands in the other half of the PSUM partition
range). Attention processes head PAIRS with 512-wide q-chunks: one score
PSUM tile holds both heads' scores for a k-tile so a single (strided)
activation computes exp for the pair.  The instruction stream is software-
pipelined: the exp-bound attention windows are back-filled with projection
and out-projection matmuls drained from a filler queue, so the PE stays
busy end to end.
"""

import numpy as np
from collections import deque
from contextlib import ExitStack

import ml_dtypes
import concourse.bass as bass
import concourse.bacc as bacc
import concourse.tile as tile
import concourse.mybir as mybir
from concourse.bass_utils import run_bass_kernel_spmd

F32 = mybir.dt.float32
BF16 = mybir.dt.bfloat16
AF = mybir.ActivationFunctionType

B = 2
S = 2048
D = 1024
DC = 256  # head dims per core (4 heads x 64)
N_CORES = 8
NT = D // 128  # 8 input-dim tiles
ST = S // 128  # 16 sequence tiles
QC = 512  # attention q-chunk
NQC = S // QC  # 4 q-chunks


def _build():
    nc = bacc.Bacc("TRN2", target_bir_lowering=False, debug=False,
                   num_devices=N_CORES)
    xt = nc.dram_tensor("xt", [D, S], BF16, kind="ExternalInput").ap()
    wq = nc.dram_tensor("wq", [D, DC], BF16, kind="ExternalInput").ap()
    wk = nc.dram_tensor("wk", [D, DC], BF16, kind="ExternalInput").ap()
    wv = nc.dram_tensor("wv", [D, DC], BF16, kind="ExternalInput").ap()
    wo = nc.dram_tensor("wo", [DC, D], BF16, kind="ExternalInput").ap()
    mk = nc.dram_tensor("mk", [128, 128], BF16, kind="ExternalInput").ap()
    y = nc.dram_tensor("y", [S, D], BF16, kind="ExternalOutput").ap()

    with tile.TileContext(nc) as tc, ExitStack() as stk:
        persist = stk.enter_context(tc.tile_pool(name="persist", bufs=1))
        # Q^T / K^T: j-block (heads 2j, 2j+1) at cols 2048j; head 2j on
        # partitions 0:64, head 2j+1 on 64:128.
        qt_sb = persist.tile([128, 2 * S], BF16)
        kt_sb = persist.tile([128, 2 * S], BF16)
        # V per k-tile block of 512 cols: head h sub-block of 128 cols =
        # [V_h | ones] for even h, [ones | V_h] for odd h.
        v_sb = persist.tile([128, ST * 512], BF16)
        ct_sb = persist.tile([128, 2 * S], BF16)   # normalized ctx^T
        wo_sb = persist.tile([128, 2 * D], BF16)   # W_o slice: d-tile at 1024d
        mk_sb = persist.tile([128, 128], BF16)     # mask[k, q] = (k <= q)
        xt_sb = persist.tile([128, NT * S], BF16)  # x^T: d-tile i at cols 2048i
        wq_sb = persist.tile([128, NT * DC], BF16)
        wk_sb = persist.tile([128, NT * DC], BF16)
        wv_sb = persist.tile([128, NT * DC], BF16)

        # ---- input DMAs (arrival order matters) ----
        def w_in(dst, src):
            nc.sync.dma_start(
                out=dst.rearrange("p (i c) -> p i c", c=DC),
                in_=src.rearrange("(i p) c -> p i c", p=128))

        xt3 = xt.rearrange("(i p) s -> p i s", p=128)
        xs3 = xt_sb.rearrange("p (i s) -> p i s", s=S)
        # minimum prefix for S1 first: W_k/W_q + x^T s-cols 0:512 (a full
        # weight costs the same DMA time as its j0 half: the half's 256-byte
        # runs pay the sub-512B 2x penalty)
        w_in(wk_sb, wk)
        for i0 in range(0, 8, 2):
            nc.sync.dma_start(out=xs3[:, i0:i0 + 2, 0:512],
                              in_=xt3[:, i0:i0 + 2, 0:512])
        w_in(wq_sb, wq)
        w_in(wv_sb, wv)
        nc.sync.dma_start(out=mk_sb[:], in_=mk[:, :])
        nc.sync.dma_start(out=xs3[:, :, 512:1024], in_=xt3[:, :, 512:1024])
        nc.sync.dma_start(out=xs3[:, :, 1024:2048], in_=xt3[:, :, 1024:2048])
        for d in range(2):
            nc.sync.dma_start(out=wo_sb[:, 1024 * d:1024 * (d + 1)],
                              in_=wo[128 * d:128 * (d + 1), :])

        # ones columns of v_sb (static): cols 64:192 of each 256 sub-block
        v3 = v_sb.rearrange("p (n c) -> p n c", c=256)
        scr_sb = persist.tile([128, 128], BF16)  # warm-up matmul scratch
        nc.vector.memset(scr_sb[:], 0.5)
        nc.vector.memset(v3[:, :, 64:192], 1.0)

        pp = stk.enter_context(tc.tile_pool(name="pp", bufs=2, space="PSUM"))
        sp = cp = op = pp
        wp = stk.enter_context(tc.tile_pool(name="wp", bufs=5))
        ep = rp = cs = ob = wp

        nid = [0]

        def tag(p):
            nid[0] += 1
            return f"{p}{nid[0]}"

        def warm_pe(n):
            """n throwaway matmuls on scratch data: keeps the PE busy (and
            its p-state ramp warm) across windows where real matmuls are
            blocked on DMA or on a cross-engine chain."""
            ps = sp.tile([128, 1024], F32, tag="s", name=tag("wm"))
            for _ in range(n):
                nc.tensor.matmul(ps[:, 0:128], lhsT=scr_sb[:, 0:128],
                                 rhs=scr_sb[:, 0:128], start=True, stop=True)

        # ---- projection group emitters (generators; yield = filler step) --
        def qk_group(w_sb, dst, j, sc, big, scalar_copy):
            """Q^T/K^T out rows = dq (j-block), s-cols [512sc, 512sc+w).
            big: [128,1024] psum on tag 's' (pre-attention, w=1024);
            else [128,512] on tag 'o' (filler, w=512)."""
            if big:
                ps = sp.tile([128, 1024], F32, tag="s", name=tag("pq"))
                cols = ((0, 512), (512, 1024))
            else:
                ps = op.tile([128, 512], F32, tag="o", name=tag("pq"))
                cols = ((0, 512),)
            for i in range(NT):
                for a, b in cols:
                    nc.tensor.matmul(
                        ps[:, a:b],
                        lhsT=w_sb[:, DC * i + 128 * j:DC * i + 128 * (j + 1)],
                        rhs=xt_sb[:, S * i + 512 * sc + a:
                                  S * i + 512 * sc + b],
                        start=(i == 0), stop=(i == NT - 1))
                if i % 4 == 3:
                    yield
            w = 1024 if big else 512
            dcol = 2048 * j + 512 * sc
            if scalar_copy:
                nc.scalar.activation(dst[:, dcol:dcol + w], ps[:, 0:w],
                                     AF.Copy)
            else:
                nc.vector.tensor_copy(dst[:, dcol:dcol + w], ps[:, 0:w])
            yield

        def v_group(st, scalar_copy):
            """V block st: out rows = s (128 of st), cols = 256 head dims."""
            ps = op.tile([128, 512], F32, tag="o", name=tag("pv"))
            for i in range(NT):
                nc.tensor.matmul(
                    ps[:, 0:256],
                    lhsT=xt_sb[:, S * i + 128 * st:S * i + 128 * (st + 1)],
                    rhs=wv_sb[:, DC * i:DC * (i + 1)],
                    start=(i == 0), stop=(i == NT - 1))
                if i % 4 == 3:
                    yield
            blk = v3[:, 2 * st:2 * st + 2, :]
            srcv = ps[:, 0:256].rearrange("p (h c) -> p h c", c=128)
            if scalar_copy:
                nc.scalar.activation(blk[:, :, 0:64], srcv[:, :, 0:64],
                                     AF.Copy)
                nc.scalar.activation(blk[:, :, 192:256], srcv[:, :, 64:128],
                                     AF.Copy)
            else:
                nc.vector.tensor_copy(blk[:, :, 0:64], srcv[:, :, 0:64])
                nc.vector.tensor_copy(blk[:, :, 192:256], srcv[:, :, 64:128])
            yield

        def oproj_group(st, half, scalar_copy, o_sb, tg="o", dsplit=False):
            """Out-projection for s-tile st, y-cols half*512.  dsplit:
            yield between the two contraction halves so the d0 matmul
            (gated only by pair-0's ctx) can run during the bridge."""
            ps = op.tile([128, 512], F32, tag=tg, name=tag("po"))
            for d in range(2):
                nc.tensor.matmul(
                    ps[:, 0:512],
                    lhsT=ct_sb[:, 2048 * d + 128 * st:
                               2048 * d + 128 * (st + 1)],
                    rhs=wo_sb[:, 1024 * d + 512 * half:
                              1024 * d + 512 * (half + 1)],
                    start=(d == 0), stop=(d == 1))
                if dsplit and d == 0:
                    yield
            yield
            if scalar_copy:
                nc.scalar.activation(o_sb[:, 512 * half:512 * (half + 1)],
                                     ps[:, 0:512], AF.Copy)
            else:
                nc.vector.tensor_copy(o_sb[:, 512 * half:512 * (half + 1)],
                                      ps[:, 0:512])
            yield

        def oproj_st(st, scalar_copy, tg="o", dsplit=False):
            o_sb = ob.tile([128, 1024], BF16, tag="ob", name=tag("ob"))
            yield from oproj_group(st, 0, scalar_copy, o_sb, tg, dsplit)
            yield from oproj_group(st, 1, scalar_copy, o_sb, tg, dsplit)
            nc.sync.dma_start(out=y[128 * st:128 * (st + 1), :], in_=o_sb[:])
            yield

        def oproj_big(st, scalar_copy, split=False):  # always d-split
            """Tail-era out-projection: full s-tile on the (now free) 's'
            psum ring.  split: halve the copy across ACT+DVE and the DMA
            across the SP+Pool queues to shorten the final drain."""
            ps = sp.tile([128, 1024], F32, tag="s", name=tag("pt"))
            o_sb = ob.tile([128, 1024], BF16, tag="ob", name=tag("obt"))
            for d in range(2):
                for a, b in ((0, 512), (512, 1024)):
                    nc.tensor.matmul(
                        ps[:, a:b],
                        lhsT=ct_sb[:, 2048 * d + 128 * st:
                                   2048 * d + 128 * (st + 1)],
                        rhs=wo_sb[:, 1024 * d + a:1024 * d + b],
                        start=(d == 0), stop=(d == 1))
                if d == 0:
                    yield
            yield
            row = y[128 * st:128 * (st + 1), :]
            if split:
                nc.scalar.activation(o_sb[:, 0:512], ps[:, 0:512], AF.Copy)
                nc.vector.tensor_copy(o_sb[:, 512:1024], ps[:, 512:1024])
                yield
                nc.sync.dma_start(out=row[:, 0:512], in_=o_sb[:, 0:512])
                nc.sync.dma_start(out=row[:, 512:1024],
                                  in_=o_sb[:, 512:1024])
                yield
                return
            if scalar_copy:
                nc.scalar.activation(o_sb[:], ps[:], AF.Copy)
            else:
                nc.vector.tensor_copy(o_sb[:], ps[:])
            yield
            nc.sync.dma_start(out=row[:], in_=o_sb[:])
            yield

        fillers = deque()
        reserve = [0]  # generators held back for the post-attention window

        def drain(n=1):
            for _ in range(n):
                if len(fillers) <= reserve[0]:
                    return
                advanced = False
                while len(fillers) > reserve[0] and not advanced:
                    try:
                        next(fillers[0])
                        advanced = True
                    except StopIteration:
                        fillers.popleft()
                if not advanced:
                    return

        def run_now(gen):
            for _ in gen:
                pass

        # ---- attention for a head pair on one q-chunk ----
        def attention(pair, qc, ndrain=1, direct_norm=False):
            """pair: 0 -> heads 0,1 (j-block 0); 1 -> heads 2,3 (j-block 1).
            q-chunk = [QC*qc, QC*(qc+1))."""
            jb = 2048 * pair
            qb = jb + QC * qc
            kt_max = 4 * qc + 3
            ctx = [cp.tile([128, 512], F32, tag="c", name=tag(f"cx{h}"))
                   for h in range(2)]
            for kt in range(kt_max + 1):
                off = max(0, 128 * kt - QC * qc)
                s_ps = sp.tile([128, 1024], F32, tag="s", name=tag("s"))
                e_sb = ep.tile([128, 1024], BF16, tag="e", bufs=8, name=tag("e"))
                for h in range(2):
                    hb = 64 * h
                    nc.tensor.matmul(
                        s_ps[:, 512 * h + off:512 * (h + 1)],
                        lhsT=kt_sb[hb:hb + 64,
                                   jb + 128 * kt:jb + 128 * (kt + 1)],
                        rhs=qt_sb[hb:hb + 64, qb + off:qb + QC],
                        start=True, stop=True)
                s3 = s_ps.rearrange("p (h c) -> p h c", c=512)
                e3 = e_sb.rearrange("p (h c) -> p h c", c=512)
                nc.scalar.activation(e3[:, :, off:512], s3[:, :, off:512],
                                     AF.Exp)
                if kt >= 4 * qc:
                    # diagonal block: zero strictly-lower (k > q).  In the
                    # final chunk the masks go to the Pool engine so the DVE
                    # is clear for the tail-critical normalize chain.
                    meng = nc.gpsimd if direct_norm else nc.vector
                    for h in range(2):
                        meng.tensor_mul(e3[:, h, off:off + 128],
                                        e3[:, h, off:off + 128],
                                        mk_sb[:, :])
                # fillers between exp and AV: they execute inside the
                # exp latency window, right where the PE would stall
                drain(ndrain if kt < kt_max else 0)
                for h in range(2):
                    nc.tensor.matmul(
                        ctx[h][:, off:512],
                        lhsT=v_sb[:, 512 * kt + 128 * (2 * pair + h):
                                  512 * kt + 128 * (2 * pair + h + 1)],
                        rhs=e3[:, h, off:512],
                        start=(kt == 0), stop=(kt == kt_max))

            # normalize: copy ctx+den to SBUF (frees PSUM), then
            # ct[hb rows] = ctx * (1/den) with den broadcast to hb rows.
            # direct_norm (last q-chunk): read straight from psum (tiles are
            # never reused), phase-order the two heads' chains, and split
            # the muls by column so the first out-proj tiles unblock early.
            if direct_norm:
                rc = []
                for h in range(2):
                    hb = 64 * h
                    dr = 64 - hb
                    rcp = rp.tile([128, 512], F32, tag="r", name=tag(f"r{h}"))
                    rcb = rp.tile([128, 512], F32, tag="rb",
                                  name=tag(f"rb{h}"))
                    # write the reciprocal to partition 0 regardless of which
                    # rows hold the denominator, so the (fast) gpsimd
                    # partition broadcast serves both heads
                    nc.vector.reciprocal(rcp[0:1, :], ctx[h][dr:dr + 1, :])
                    nc.gpsimd.partition_broadcast(rcb[:, :], rcp[0:1, :])
                    rc.append(rcb)
                for a in (0, 256):
                    for h in range(2):
                        hb = 64 * h
                        nc.vector.tensor_mul(
                            ct_sb[hb:hb + 64, qb + a:qb + a + 256],
                            ctx[h][hb:hb + 64, a:a + 256],
                            rc[h][hb:hb + 64, a:a + 256])
                return
            for h in range(2):
                hb = 64 * h          # ctx rows for this head in its psum tile
                dr = 64 - hb         # denominator rows
                cd = cs.tile([128, 512], F32, tag="cd", name=tag(f"cd{h}"))
                nc.vector.tensor_copy(cd[:, :], ctx[h][:, :])
                rcp = rp.tile([128, 512], F32, tag="r", name=tag(f"r{h}"))
                rcb = rp.tile([128, 512], F32, tag="rb", name=tag(f"rb{h}"))
                nc.vector.reciprocal(rcp[dr:dr + 1, :], cd[dr:dr + 1, :])
                if dr == 0:
                    # gpsimd broadcast (reads true partition 0 only)
                    nc.gpsimd.partition_broadcast(rcb[:, :], rcp[0:1, :])
                else:
                    # issued from the Pool queue so a wait on the reciprocal
                    # can't head-of-line-block the SP queue's y writebacks
                    nc.gpsimd.dma_start(
                        out=rcb[hb:hb + 64, :],
                        in_=rcp[dr:dr + 1, :].unsqueeze(1)
                        .to_broadcast((1, 64, 512)))
                nc.vector.tensor_mul(
                    ct_sb[hb:hb + 64, qb:qb + QC],
                    cd[hb:hb + 64, :], rcb[hb:hb + 64, :])
                drain(ndrain)

        # ================= emission schedule =================
        # ramp the PE p-state while the first DMAs land
        warm_pe(22)
        # S1: minimum prefix for (pair 0, qc 0): K/Q j0 s0:512, V st0..3.
        run_now(qk_group(wk_sb, kt_sb, 0, 0, False, True))  # K j0 s0:512
        run_now(qk_group(wq_sb, qt_sb, 0, 0, False, True))  # Q j0 s0:512
        for st in range(4):
            run_now(v_group(st, True))

        # registry of remaining projection groups (also queued as fillers).
        kq_gen = {}
        for j in range(2):
            for sc in range(4):
                if j == 0 and sc == 0:
                    continue
                kq_gen[("k", j, sc)] = qk_group(wk_sb, kt_sb, j, sc,
                                                False, False)
                kq_gen[("q", j, sc)] = qk_group(wq_sb, qt_sb, j, sc,
                                                False, False)
        v_gen = {st: v_group(st, False) for st in range(4, 16)}

        # priority order: j1 s0:512 (pair-1 qc0), then the s512:1024
        # chunks (qc1), V st4..7 (qc1), then s1024:2048, then V st8..15.
        for key in (("k", 1, 0), ("q", 1, 0),
                    ("k", 0, 1), ("q", 0, 1), ("k", 1, 1), ("q", 1, 1)):
            fillers.append(kq_gen[key])
        for st in range(4, 8):
            fillers.append(v_gen[st])
        for sc in (2, 3):
            for j in range(2):
                fillers.append(kq_gen[("k", j, sc)])
                fillers.append(kq_gen[("q", j, sc)])
        for st in range(8, 16):
            fillers.append(v_gen[st])

        def need(pair, qc):
            """Force-emit everything attention(pair, qc) reads."""
            j = pair
            for sc in range(qc + 1):
                if (("k", j, sc)) in kq_gen:
                    run_now(kq_gen[("k", j, sc)])
            if ("q", j, qc) in kq_gen:
                run_now(kq_gen[("q", j, qc)])
            for st in range(4 * qc + 4):
                if st in v_gen:
                    run_now(v_gen[st])

        # fillers are drained during attention; two qc2 out-proj groups are
        # held out (ACT copies) to bridge the last normalize chains without
        # touching the DVE, keeping the PE busy and its p-state warm.
        late = []
        for qc in range(NQC):
            # during qc1, shield the (just queued) qc0 out-proj groups so
            # they land in the filler-starved qc2/qc3 windows instead
            reserve[0] = 4 if qc == 1 else 0
            for pair in range(2):
                need(pair, qc)
                attention(pair, qc, ndrain=(2 if qc < 2 else 3),
                          direct_norm=(qc == NQC - 1 and pair == 1))
            if qc < NQC - 1:
                for st in range(4 * qc, 4 * qc + 4):
                    if qc == 2 and st >= 10:
                        late.append(oproj_st(st, True))
                    else:
                        fillers.append(oproj_st(st, False))

        # bridge the last normalize chains: keep the PE warm so the tail
        # matmuls run at full clock
        warm_pe(20)
        # tail: held-back groups first, then the last q-chunk's
        # out-projection on the now-free 's' psum ring.
        tail = deque(late)
        tail.append(oproj_st(12, True, "o"))
        tail.append(oproj_big(13, True))
        tail.append(oproj_big(14, False))
        tail.append(oproj_st(15, False, "c"))
        while fillers or tail:
            drain(1)
            if tail:
                g = tail.popleft()
                try:
                    next(g)
                    tail.append(g)
                except StopIteration:
                    pass

    nc.compile()
    return nc


_nc = None


def make_in_maps(x, W_q, W_k, W_v, W_o):
    """Per-core input dict construction (shared with test.py)."""
    bf = ml_dtypes.bfloat16
    mask = np.triu(np.ones((128, 128), dtype=np.float32))  # 1 where k <= q
    in_maps = []
    for c in range(N_CORES):
        b = c // 4
        g = c % 4
        sl = slice(DC * g, DC * (g + 1))
        in_maps.append({
            "xt": np.ascontiguousarray(x[b].T).astype(bf),
            "wq": (np.ascontiguousarray(W_q[:, sl]) * 0.125).astype(bf),
            "wk": np.ascontiguousarray(W_k[:, sl]).astype(bf),
            "wv": np.ascontiguousarray(W_v[:, sl]).astype(bf),
            "wo": np.ascontiguousarray(W_o[sl, :]).astype(bf),
            "mk": mask.astype(bf),
        })
    return in_maps


def kernel(x, W_q, W_k, W_v, W_o, b_o):
    global _nc
    x = np.ascontiguousarray(np.asarray(x, dtype=np.float32))
    W_q = np.asarray(W_q, dtype=np.float32)
    W_k = np.asarray(W_k, dtype=np.float32)
    W_v = np.asarray(W_v, dtype=np.float32)
    W_o = np.asarray(W_o, dtype=np.float32)
    b_o = np.asarray(b_o, dtype=np.float32)

    if _nc is None:
        _nc = _build()

    in_maps = make_in_maps(x, W_q, W_k, W_v, W_o)
    res = run_bass_kernel_spmd(_nc, in_maps, list(range(N_CORES)))
    parts = [res.results[c]["y"] for c in range(N_CORES)]
    out = np.empty((B, S, D), dtype=np.float32)
    for b in range(B):
        acc = np.zeros((S, D), dtype=np.float64)
        for g in range(4):
            acc += np.asarray(parts[4 * b + g], dtype=np.float32)
        acc += b_o
        out[b] = acc.astype(np.float32)
    return out
